# revision 11
# baseline (speedup 1.0000x reference)
"""Multi-Head Latent Attention (MLA) TRN2 Bass kernel.

Sharding: data-parallel over batch (B=2) x tensor-parallel over heads
(16 heads -> 4 per core) = 8 cores. The kv_lora latent path and shared
rope key are computed replicated within each batch group (cross-core
AllGather / Pool-engine offload both trip the chip power throttle and
net out slower -- measured in a previous session); the final output
projection is computed as per-core partials which the host sums.

Fully software-pipelined round structure (one round per 512-column
quarter t of the sequence):
  round t: prefetch x(t+1) | norm-flush + out-proj(t-1) | kv_a(t) bf16 |
           q-proj(t) fp8 DoubleRow | kv_b(t) | RoPE(t) + fp8 pack(t) |
           attention units (h=0..3, j=t)
so the ACT-heavy attention exp work of round t overlaps the PE-heavy
projection work of round t+1, and the output DMA is spread across the
whole run instead of draining at the end.

All on-device dataflow is "transposed" (feature dim on partitions,
sequence on the free dim) so no PE transposes are ever needed. Scores
are computed per (head, q-block of 512) via a single fp8e4 DoubleRow
matmul per k-chunk (the 192-dim nope+rope contraction packed as 2x96
partitions, q/k pre-scaled by 64/32 and descaled inside the exp), causal
masks added on diagonal chunks by a bf16 PE matmul, exp on ACT (no max
subtraction; scores are bounded), softmax denominators via running
elementwise bf16 sums of the exp'd chunks on the DVE + one ones-matmul
per unit, outT accumulated in PSUM and normalized by broadcasted
reciprocals.

PSUM budget (8 banks): a single 2-deep ring of [128, 1024] tiles (tag
"s", 4 banks) is timeshared in PE program order by the phase-1
accumulator pairs, kv_b pairs, score pair tiles, the norm broadcast, and
out-proj pairs; 2 banks for the attention out accumulators; 1 bank for
denominators; 1 bank for the rope/q4/q5 accumulator.

x and outp use chunk-major DRAM layouts so every DMA is one dense
contiguous block (host packs/unpacks); outp is bf16 partials summed in
f32 on the host.
"""

import math
import sys

import numpy as np
import ml_dtypes

try:  # concourse ships in the container; fall back to the repo checkout
    import concourse.bass  # noqa: F401
except ImportError:  # pragma: no cover
    for p in ("/opt/trn_rl_repo", "/root/.axon_site/_ro/trn_rl_repo"):
        if p not in sys.path:
            sys.path.insert(0, p)

# Problem constants (hardcoded; harness calls kernel() standalone).
D_MODEL = 2048
N_HEADS = 16
R = 512          # kv lora rank
DN = 128         # d_nope
DR = 64          # d_rope
DV = 128         # d_v
ROPE_THETA = 10000.0
B = 2
S = 2048
HP = 4           # heads per core
QB = 512         # q block size
NKC = S // 128   # 16 k chunks
NQB = S // QB    # 4 q blocks
NCORES = 8

BF16 = ml_dtypes.bfloat16
F8E4 = ml_dtypes.float8_e4m3  # mybir float8e4 (IEEE e4m3, max finite 240)

# fp8 pre-scales. QS/KS: q/k tiles feeding the score matmuls (descaled
# inside the exp). SXQ/SWQ: host-side scales for x / Wq feeding the fp8
# DoubleRow q-projection (descaled in the PSUM->SBUF store).
QS, KS = 64.0, 32.0
SXQ, SWQ = 16.0, 8192.0

_PROGRAM = {}


def _build_program(split_waits=True):
    import concourse.bass as bass
    import concourse.mybir as mybir
    from concourse.tile import TileContext

    def split_multi_waits(max_waits=1):
        """The walrus build in this container rejects instructions with
        more than `max_waits` sync-wait commands. Move excess waits onto
        same-engine NoOps inserted just before the instruction."""
        for f in nc.m.functions:
            for bb in f.blocks:
                out = []
                changed = False
                for inst in bb.instructions:
                    si = getattr(inst, "sync_info", None)
                    ws = list(si.on_wait) if si is not None else []
                    if len(ws) > max_waits:
                        changed = True
                        inst.sync_info = mybir.SyncInfo(
                            on_wait=ws[:max_waits],
                            on_update=list(si.on_update))
                        for w in ws[max_waits:]:
                            n = mybir.InstNoOp(
                                name=nc.get_next_instruction_name(),
                                ins=[], outs=[])
                            n.engine = inst.engine
                            n.sync_info = mybir.SyncInfo(
                                on_wait=[w], on_update=[])
                            out.append(n)
                    out.append(inst)
                if changed:
                    bb.instructions = out

    f32 = mybir.dt.float32
    cdt = mybir.dt.bfloat16
    f8 = mybir.dt.float8e4
    DRmode = mybir.MatmulPerfMode.DoubleRow
    Copy = mybir.ActivationFunctionType.Copy
    Exp = mybir.ActivationFunctionType.Exp
    Ln = mybir.ActivationFunctionType.Ln

    nc = bass.Bass()

    # x arrives chunk-major: block (t, k) = xT_logical[128k:128k+128,
    # 512t:512t+512] stored contiguously so every chunk DMA is one dense
    # 128KB transfer instead of 128 separate 1KB rows.
    xT = nc.dram_tensor("xT", [NQB * 16 * 128, QB], cdt, kind="ExternalInput")
    # fp8 copy of x for the q-projection, pair-major for DoubleRow:
    # block (t, p) = [128, 2, 512]: elem (kp, j, col) =
    # SXQ * x[512t+col, 256p+128j+kp] (feature on partitions).
    xT8 = nc.dram_tensor("xT8", [NQB * 8 * 128, 2 * QB], f8,
                         kind="ExternalInput")
    # Wq in fp8, pair-major: block p = [128, 2, 768]: elem (kp, j, m) =
    # SWQ * scale * Wq[256p+128j+kp, m] (m = head-major nope|rope cols).
    wq8 = nc.dram_tensor("wq8", [8 * 128, 2 * HP * (DN + DR)], f8,
                         kind="ExternalInput")
    wkva = nc.dram_tensor("wkva", [D_MODEL, R + DR], cdt, kind="ExternalInput")
    wkvbk = nc.dram_tensor("wkvbk", [R, HP * DN], cdt, kind="ExternalInput")
    wkvbv = nc.dram_tensor("wkvbv", [R, HP * DV], cdt, kind="ExternalInput")
    wo = nc.dram_tensor("wo", [HP * DV, D_MODEL], cdt, kind="ExternalInput")
    cosf = nc.dram_tensor("cosf", [128, S], cdt, kind="ExternalInput")
    sinf = nc.dram_tensor("sinf", [128, S], cdt, kind="ExternalInput")
    masks = nc.dram_tensor("masks", [128, 128], cdt, kind="ExternalInput")
    ident = nc.dram_tensor("ident", [128, 128], cdt, kind="ExternalInput")
    ones = nc.dram_tensor("ones", [128, 1], cdt, kind="ExternalInput")
    onesf = nc.dram_tensor("onesf", [1, 128], cdt, kind="ExternalInput")
    # outp is block-major: block (m, nb) stored contiguously; host unpacks.
    outp = nc.dram_tensor("outp", [16 * NQB * 128, QB], cdt,
                          kind="ExternalOutput")

    with TileContext(nc) as tc:
        with (
            tc.tile_pool(name="const", bufs=1) as cpool,
            tc.tile_pool(name="persist", bufs=1) as ppool,
            tc.tile_pool(name="wproj", bufs=1) as wpool,
            tc.tile_pool(name="xstream", bufs=1) as xpool,
            tc.tile_pool(name="att", bufs=1) as apool,
            tc.tile_pool(name="psS", bufs=2, space="PSUM") as psS,
            tc.tile_pool(name="psO", bufs=2, space="PSUM") as psO,
            tc.tile_pool(name="psD", bufs=1, space="PSUM") as psD,
            tc.tile_pool(name="psX", bufs=1, space="PSUM") as psX,
        ):
            cosf_sb = cpool.tile([128, S], cdt, name="cosf_sb")
            sinf_sb = cpool.tile([128, S], cdt, name="sinf_sb")
            masks_sb = cpool.tile([128, 128], cdt, name="masks_sb")
            ident_sb = cpool.tile([128, 128], cdt, name="ident_sb")
            ones_sb = cpool.tile([128, 1], cdt, name="ones_sb")
            onesb_sb = cpool.tile([1, 128], cdt, name="onesb_sb")

            # Persistent activations: only what later rounds consume.
            # DoubleRow-packed per-head q/k tensors: 192 contraction dims
            # as 2 blocks of 96 partitions (blk0 = dims 0:96, blk1 =
            # dims 96:192 = nope 96:128 + rope 0:64).
            qf8 = [
                ppool.tile([96, NQB, 2, QB], f8, name=f"qf8_{h}", tag="qf8",
                           bufs=4)
                for h in range(HP)
            ]
            kf8 = [
                ppool.tile([96, NKC, 2, 128], f8, name=f"kf8_{h}", tag="kf8",
                           bufs=4)
                for h in range(HP)
            ]
            vt = [
                ppool.tile([128, HP * DV], cdt, name=f"v{i}", tag="v",
                           bufs=NKC)
                for i in range(NKC)
            ]
            outT = [
                ppool.tile([128, S], cdt, name=f"outT{h}", tag="outT",
                           bufs=4)
                for h in range(HP)
            ]

            # Weights persist; DMAs issue at startup on the scalar queue.
            wbk_sb = [
                ppool.tile([128, HP * DN], cdt, name=f"wbk_sb{r}", tag="wbk",
                           bufs=4)
                for r in range(4)
            ]
            wbv_sb = [
                ppool.tile([128, HP * DV], cdt, name=f"wbv_sb{r}", tag="wbv",
                           bufs=4)
                for r in range(4)
            ]
            wo_sb = [
                ppool.tile([128, D_MODEL], cdt, name=f"wo_sb{r}", tag="wo",
                           bufs=4)
                for r in range(4)
            ]

            wkva_sb = []
            wq8_sb = []
            for k in range(16):
                w2 = wpool.tile([128, R + DR], cdt, name=f"wkva_sb{k}",
                                tag="wkva", bufs=16)
                nc.scalar.dma_start(w2, wkva[k * 128:(k + 1) * 128, :])
                wkva_sb.append(w2)
                if k < 8:
                    w1 = wpool.tile([128, 2, HP * (DN + DR)], f8,
                                    name=f"wq8_sb{k}", tag="wq8", bufs=8)
                    nc.scalar.dma_start(w1, wq8[k * 128:(k + 1) * 128, :])
                    wq8_sb.append(w1)
            for r in range(4):
                nc.scalar.dma_start(wbk_sb[r], wkvbk[r * 128:(r + 1) * 128, :])
                nc.scalar.dma_start(wbv_sb[r], wkvbv[r * 128:(r + 1) * 128, :])
            nc.scalar.dma_start(cosf_sb, cosf[:, :])
            nc.scalar.dma_start(sinf_sb, sinf[:, :])
            nc.scalar.dma_start(masks_sb, masks[:, :])
            nc.scalar.dma_start(ident_sb, ident[:, :])
            nc.scalar.dma_start(ones_sb, ones[:, :])
            nc.scalar.dma_start(onesb_sb, onesf[:, :])
            for r in range(4):
                nc.scalar.dma_start(wo_sb[r], wo[r * 128:(r + 1) * 128, :])

            def emit_x_dmas(t):
                xq = []
                xq8t = []
                for k in range(16):
                    xk = xpool.tile([128, QB], cdt, name=f"xq_{t}_{k}",
                                    tag="xq", bufs=18)
                    nc.sync.dma_start(
                        xk, xT[(t * 16 + k) * 128:(t * 16 + k + 1) * 128, :])
                    xq.append(xk)
                    if k % 2 == 1:
                        p = k // 2
                        x8 = xpool.tile([128, 2, QB], f8,
                                        name=f"xq8_{t}_{p}", tag="xq8",
                                        bufs=9)
                        nc.sync.dma_start(
                            x8,
                            xT8[(t * 8 + p) * 128:(t * 8 + p + 1) * 128, :])
                        xq8t.append(x8)
                return xq, xq8t

            def stile(name):
                return psS.tile([128, 2 * QB], f32, name=name, tag="s",
                                bufs=2)

            def emit_phase1(t, xq, xq8t):
                """kv_a (bf16) + q-proj (fp8 DR) for quarter t. Returns
                the per-quarter scratch tiles (ck, kr, qT8, qTr, qr8,
                kr8 slots are allocated here)."""
                ckS = [
                    ppool.tile([128, QB], cdt, name=f"ck_{t}_{m}", tag="ck",
                               bufs=4)
                    for m in range(4)
                ]
                krq = ppool.tile([128, QB], cdt, name=f"kr_{t}", tag="krq",
                                 bufs=1)
                qT8S = [
                    ppool.tile([128, QB], f8, name=f"qT8_{t}_{m}", tag="qT8",
                               bufs=4)
                    for m in range(4)
                ]
                qTrS = [
                    ppool.tile([128, QB], cdt, name=f"qTr_{t}_{m}", tag="qTr",
                               bufs=2)
                    for m in range(2)
                ]
                # pass A: kv_a. 5 accumulators (2 pair tiles + rope in
                # the x1 bank); each landing x chunk feeds 5 matmuls.
                sA = [stile(f"pA_{t}_{i}") for i in range(2)]
                xr = psX.tile([64, QB], f32, name=f"pxr_{t}", tag="x1",
                              bufs=1)
                for k in range(16):
                    for m in range(4):
                        reg = sA[m // 2][:, (m % 2) * QB:(m % 2 + 1) * QB]
                        nc.tensor.matmul(
                            reg, lhsT=wkva_sb[k][:, m * 128:(m + 1) * 128],
                            rhs=xq[k], start=(k == 0), stop=(k == 15),
                            skip_group_check=True)
                    nc.tensor.matmul(
                        xr, lhsT=wkva_sb[k][:, R:R + DR], rhs=xq[k],
                        start=(k == 0), stop=(k == 15))
                # All phase-1 PSUM->SBUF stores run on the DVE: the ACT
                # queue carries the previous round's attention exp burst,
                # and ACT-side stores here would stall the PE's x1-ring
                # and pack progression behind it.
                for m in range(4):
                    nc.vector.tensor_copy(
                        ckS[m], sA[m // 2][:, (m % 2) * QB:(m % 2 + 1) * QB])
                nc.vector.tensor_copy(krq[0:64, :], xr)
                nc.vector.tensor_copy(krq[64:128, :], xr)
                # pass C: q nope m0..3 (pair tiles) + rope m4 (x1 bank).
                sC = [stile(f"pC_{t}_{i}") for i in range(2)]
                x4 = psX.tile([128, QB], f32, name=f"px4_{t}", tag="x1",
                              bufs=1)
                for p in range(8):
                    for m in range(4):
                        reg = sC[m // 2][:, (m % 2) * QB:(m % 2 + 1) * QB]
                        nc.tensor.matmul(
                            reg, lhsT=wq8_sb[p][:, :, m * 128:(m + 1) * 128],
                            rhs=xq8t[p], start=(p == 0), stop=(p == 7),
                            perf_mode=DRmode, skip_group_check=True)
                    nc.tensor.matmul(
                        x4, lhsT=wq8_sb[p][:, :, 4 * 128:5 * 128],
                        rhs=xq8t[p], start=(p == 0), stop=(p == 7),
                        perf_mode=DRmode)
                for m in range(4):
                    nc.vector.tensor_scalar_mul(
                        qT8S[m], sC[m // 2][:, (m % 2) * QB:(m % 2 + 1) * QB],
                        QS / (SXQ * SWQ))
                nc.vector.tensor_scalar_mul(qTrS[0], x4, 1.0 / (SXQ * SWQ))
                # pass D: rope m5 (x1 bank).
                x5 = psX.tile([128, QB], f32, name=f"px5_{t}", tag="x1",
                              bufs=1)
                for p in range(8):
                    nc.tensor.matmul(
                        x5, lhsT=wq8_sb[p][:, :, 5 * 128:6 * 128],
                        rhs=xq8t[p], start=(p == 0), stop=(p == 7),
                        perf_mode=DRmode)
                nc.vector.tensor_scalar_mul(qTrS[1], x5, 1.0 / (SXQ * SWQ))
                return ckS, krq, qT8S, qTrS

            def emit_kvb(t, ckS):
                """k_nope + v up-projection for quarter t."""
                kn8S = [
                    ppool.tile([128, QB], f8, name=f"kn8_{t}_{m}", tag="kn8",
                               bufs=4)
                    for m in range(4)
                ]
                sK = [stile(f"pK_{t}_{i}") for i in range(2)]
                for m in range(4):
                    reg = sK[m // 2][:, (m % 2) * QB:(m % 2 + 1) * QB]
                    for r in range(4):
                        nc.tensor.matmul(
                            reg, lhsT=wbk_sb[r][:, m * 128:(m + 1) * 128],
                            rhs=ckS[r], start=(r == 0), stop=(r == 3),
                            skip_group_check=True)
                    nc.vector.tensor_scalar_mul(kn8S[m], reg, KS)
                sV = [stile(f"pV_{t}_{i}") for i in range(2)]
                for ci in range(4):
                    c = 4 * t + ci
                    reg = sV[ci // 2][:, (ci % 2) * QB:(ci % 2 + 1) * QB]
                    for r in range(4):
                        nc.tensor.matmul(
                            reg, lhsT=ckS[r][:, ci * 128:(ci + 1) * 128],
                            rhs=wbv_sb[r], start=(r == 0), stop=(r == 3),
                            skip_group_check=True)
                    nc.vector.tensor_copy(vt[c], reg)
                return kn8S

            def emit_rope_pack(t, krq, qT8S, qTrS, kn8S):
                """RoPE rotation for quarter t + fp8 packing of the
                DoubleRow q/k tensors."""
                qr8S = [
                    ppool.tile([128, QB], f8, name=f"qr8_{t}_{m}", tag="qr8",
                               bufs=2)
                    for m in range(2)
                ]
                kr8S = ppool.tile([128, QB], f8, name=f"kr8_{t}", tag="kr8",
                                  bufs=1)
                tcols = slice(t * QB, (t + 1) * QB)
                # rot = x * cosf + swap32(x) * sinf (signs folded in sinf).
                for idx, tap in enumerate([qTrS[0], qTrS[1], krq]):
                    sw = ppool.tile([128, QB], cdt, name=f"sw_{t}_{idx}",
                                    tag="sw", bufs=3)
                    for blk in range(4):
                        src = (blk ^ 1) * 32
                        nc.sync.dma_start(
                            sw[blk * 32:(blk + 1) * 32, :],
                            tap[src:src + 32, :])
                    nc.vector.tensor_mul(tap, tap, cosf_sb[:, tcols])
                    nc.vector.tensor_mul(sw, sw, sinf_sb[:, tcols])
                    nc.vector.tensor_add(tap, tap, sw)
                nc.vector.tensor_scalar_mul(qr8S[0], qTrS[0], QS)
                nc.vector.tensor_scalar_mul(qr8S[1], qTrS[1], QS)
                nc.vector.tensor_scalar_mul(kr8S, krq, KS)
                c4 = slice(4 * t, 4 * t + 4)
                for h in range(HP):
                    off = (h % 2) * 64
                    ri = h // 2
                    nc.sync.dma_start(qf8[h][0:96, t, 0, :], qT8S[h][0:96, :])
                    nc.sync.dma_start(qf8[h][0:32, t, 1, :],
                                      qT8S[h][96:128, :])
                    nc.sync.dma_start(qf8[h][32:96, t, 1, :],
                                      qr8S[ri][off:off + 64, :])
                    nc.scalar.dma_start(kf8[h][0:96, c4, 0, :],
                                        kn8S[h][0:96, :])
                    nc.scalar.dma_start(kf8[h][0:32, c4, 1, :],
                                        kn8S[h][96:128, :])
                    nc.scalar.dma_start(kf8[h][32:96, c4, 1, :],
                                        kr8S[off:off + 64, :])

            def calc_dps(h, j, ptacc):
                # Single partition-sum matmul over the accumulated exp'd
                # chunks (deferred off the critical path).
                dps = psD.tile([1, QB], f32, name=f"dps_{h}_{j}", tag="d")
                nc.tensor.matmul(dps, lhsT=ones_sb, rhs=ptacc,
                                 start=True, stop=True)
                return dps

            def norm_early(h, j, dps):
                # 1/denom as exp(-ln(d)) on the ACT engine.
                rec = apool.tile([1, QB], f32, name=f"rec_{h}_{j}",
                                 tag="rec", bufs=2)
                nc.scalar.activation(rec, dps, Ln)
                recb = apool.tile([1, QB], cdt, name=f"recb_{h}_{j}",
                                  tag="recb", bufs=2)
                nc.scalar.activation(recb, rec, Exp, scale=-1.0)
                return recb

            def norm_late(h, j, ops, recb):
                # Broadcast 1/denom across partitions via a K=1 matmul
                # (into the first bank of an s-ring tile), then scale the
                # out accumulator into outT.
                qs = slice(j * QB, (j + 1) * QB)
                bps = stile(f"bps_{h}_{j}")[:, 0:QB]
                nc.tensor.matmul(bps, lhsT=onesb_sb, rhs=recb,
                                 start=True, stop=True,
                                 skip_group_check=True)
                bc = apool.tile([128, QB], f32, name=f"bc_{h}_{j}",
                                tag="bc", bufs=2)
                nc.scalar.copy(bc, bps)
                nc.vector.tensor_mul(outT[h][:, qs], ops, bc)

            def emit_attention_unit(h, j, pend):
                """One (head, q-block) attention unit; returns the new
                pend tuple. pend norm work for the previous unit is
                emitted at pair slots 0 (early) and 1 (late)."""
                qs = slice(j * QB, (j + 1) * QB)
                ops = psO.tile([128, QB], f32, name=f"ops_{h}_{j}", tag="o")
                ptacc = apool.tile([128, QB], cdt, name=f"pta_{h}_{j}",
                                   tag="pta", bufs=2)
                nch = 4 * (j + 1)
                # Chunk PAIRS share one s-ring tile so a single exp covers
                # both chunks -- each ACT op carries ~280ns fixed overhead.
                for p in range(nch // 2):
                    cA, cB = 2 * p, 2 * p + 1
                    rA, rB = cA - 4 * j, cB - 4 * j
                    colA = max(0, rA * 128)
                    colB = max(0, rB * 128)
                    wA = slice(colA, QB)
                    wB = slice(colB, QB)
                    pairps = stile(f"sps_{h}_{j}_{p}")
                    nc.tensor.matmul(
                        pairps[:, colA:QB],
                        lhsT=kf8[h][:, cA, :, :],
                        rhs=qf8[h][:, j, :, colA:QB],
                        start=True, stop=(rA < 0),
                        perf_mode=DRmode, skip_group_check=True)
                    if rA >= 0:
                        nc.tensor.matmul(
                            pairps[:, colA:colA + 128],
                            lhsT=ident_sb, rhs=masks_sb,
                            start=False, stop=True,
                            skip_group_check=True)
                    # The B half always writes its full [QB:2QB] range:
                    # the exp below reads [colA:2QB] contiguously, so a
                    # partial write would leave a stale-data gap at
                    # [QB:QB+colB] (the masked columns' values are real
                    # but discarded).
                    nc.tensor.matmul(
                        pairps[:, QB:2 * QB],
                        lhsT=kf8[h][:, cB, :, :],
                        rhs=qf8[h][:, j, :, 0:QB],
                        start=True, stop=(rB < 0),
                        perf_mode=DRmode, skip_group_check=True)
                    if rB >= 0:
                        nc.tensor.matmul(
                            pairps[:, QB + colB:QB + colB + 128],
                            lhsT=ident_sb, rhs=masks_sb,
                            start=False, stop=True,
                            skip_group_check=True)
                    ptp = apool.tile([128, 2 * QB], cdt,
                                     name=f"pt_{h}_{j}_{p}", tag="pt",
                                     bufs=2)
                    nc.scalar.activation(
                        ptp[:, colA:2 * QB], pairps[:, colA:2 * QB], Exp,
                        scale=1.0 / (QS * KS))
                    nc.tensor.matmul(
                        ops[:, wA], lhsT=vt[cA][:, h * DV:(h + 1) * DV],
                        rhs=ptp[:, colA:QB], start=(cA == 0),
                        stop=False, skip_group_check=True)
                    nc.tensor.matmul(
                        ops[:, wB], lhsT=vt[cB][:, h * DV:(h + 1) * DV],
                        rhs=ptp[:, QB + colB:2 * QB],
                        start=False, stop=(cB == nch - 1),
                        skip_group_check=True)
                    if cA == 0:
                        nc.vector.tensor_copy(ptacc, ptp[:, 0:QB])
                    else:
                        nc.vector.tensor_add(
                            ptacc[:, wA], ptacc[:, wA], ptp[:, colA:QB])
                    nc.vector.tensor_add(
                        ptacc[:, wB], ptacc[:, wB], ptp[:, QB + colB:2 * QB])
                    if p == 0 and pend is not None and len(pend) == 5:
                        ph, pj, pops, pacc, _ = pend
                        recb = norm_early(ph, pj, calc_dps(ph, pj, pacc))
                        pend = (ph, pj, pops, recb)
                    if p == 1 and pend is not None and len(pend) == 4:
                        ph, pj, pops, recb = pend
                        norm_late(ph, pj, pops, recb)
                        pend = None
                return (h, j, ops, ptacc, True)

            def emit_outproj(nb):
                """Output projection for column block nb (16 M-tiles as 8
                s-ring pair tiles), streaming each block out as soon as
                its staging copy lands."""
                ncols = slice(nb * QB, (nb + 1) * QB)
                for mp in range(8):
                    tile = stile(f"pw_{nb}_{mp}")
                    for i in range(2):
                        m = 2 * mp + i
                        reg = tile[:, i * QB:(i + 1) * QB]
                        for r in range(4):
                            nc.tensor.matmul(
                                reg,
                                lhsT=wo_sb[r][:, m * 128:(m + 1) * 128],
                                rhs=outT[r][:, ncols], start=(r == 0),
                                stop=(r == 3), skip_group_check=True)
                        st = apool.tile([128, QB], cdt, name=f"st_{nb}_{m}",
                                        tag="st", bufs=4)
                        if i == 0:
                            nc.scalar.copy(st, reg)
                        else:
                            nc.vector.tensor_copy(st, reg)
                        eng = nc.sync if i == 0 else nc.scalar
                        eng.dma_start(
                            outp[(m * NQB + nb) * 128:
                                 (m * NQB + nb + 1) * 128, :], st)

            # ================= the pipelined rounds =================
            pend = None
            xq, xq8t = emit_x_dmas(0)
            for t in range(NQB):
                ckS, krq, qT8S, qTrS = emit_phase1(t, xq, xq8t)
                if pend is not None:
                    # Finish the last unit of round t-1 (its dps/recb were
                    # emitted at the end of that round).
                    ph, pj, pops, recb = pend
                    norm_late(ph, pj, pops, recb)
                    pend = None
                kn8S = emit_kvb(t, ckS)
                emit_rope_pack(t, krq, qT8S, qTrS, kn8S)
                # Prefetch the next quarter's x AFTER the rope/pack DMAs
                # so those (which gate this round's attention) aren't
                # stuck behind 3MB of prefetch on the sync queue.
                nxt = emit_x_dmas(t + 1) if t + 1 < NQB else None
                if t > 0:
                    # Stream out the completed column block; its PE work
                    # also covers the pack(t) DMA latency before the
                    # first attention unit of this round.
                    emit_outproj(t - 1)
                for h in range(HP):
                    pend = emit_attention_unit(h, t, pend)
                # Kick off the last unit's denominator/reciprocal now so
                # its norm_late (next round) doesn't stall the PE.
                ph, pj, pops, pacc, _ = pend
                recb = norm_early(ph, pj, calc_dps(ph, pj, pacc))
                pend = (ph, pj, pops, recb)
                xq, xq8t = (nxt if nxt is not None else (None, None))

            # Final flush: last unit's norm + last column block.
            ph, pj, pops, recb = pend
            norm_late(ph, pj, pops, recb)
            emit_outproj(NQB - 1)

    if split_waits:
        split_multi_waits()
    return nc


def get_program(split_waits=True):
    key = (split_waits,)
    if key not in _PROGRAM:
        _PROGRAM[key] = _build_program(split_waits)
    return _PROGRAM[key]


def make_core_inputs(x, Wq, Wkv_a, Wkv_b, Wo):
    """Host-side sharding/pre-processing. Returns list of 8 input dicts."""
    scale = 1.0 / math.sqrt(DN + DR)

    inv_freq = 1.0 / (ROPE_THETA ** (np.arange(0, DR, 2, dtype=np.float64) / DR))
    t = np.arange(S, dtype=np.float64)
    freqs = np.outer(t, inv_freq)                      # [S, 32]
    cos32 = np.cos(freqs).T.astype(np.float32)         # [32, S]
    sin32 = np.sin(freqs).T.astype(np.float32)
    cosf = np.tile(cos32, (4, 1)).astype(BF16)         # [128, S]
    sinf = np.tile(np.concatenate([-sin32, sin32], axis=0), (2, 1)).astype(BF16)

    row = np.arange(128)[:, None]
    col = np.arange(128)[None, :]
    masks = np.where(col >= row, 0.0, -1e30).astype(BF16)  # [128, 128]
    ident = np.eye(128, dtype=BF16)
    ones = np.ones([128, 1], dtype=BF16)
    onesf = np.ones([1, 128], dtype=BF16)

    Wq_r = np.asarray(Wq, dtype=np.float32).reshape(D_MODEL, N_HEADS, DN + DR)
    Wb_r = np.asarray(Wkv_b, dtype=np.float32).reshape(R, N_HEADS, DN + DV)
    Wo_f = np.asarray(Wo, dtype=np.float32)
    Wkva_f = np.asarray(Wkv_a, dtype=np.float32)
    x_f = np.asarray(x, dtype=np.float32)

    in_maps = []
    for c in range(NCORES):
        b, g = divmod(c, HP)
        heads = list(range(HP * g, HP * g + HP))
        # chunk-major xT: block (t, k) contiguous [128, 512]
        xTc = np.ascontiguousarray(
            x_f[b].T.reshape(16, 128, NQB, QB).transpose(2, 0, 1, 3)
            .reshape(NQB * 16 * 128, QB)).astype(BF16)
        # fp8 pair-major x for the DoubleRow q projection.
        xT8c = np.ascontiguousarray(
            (x_f[b].T * SXQ).reshape(8, 2, 128, NQB, QB)
            .transpose(3, 0, 2, 1, 4)
            .reshape(NQB * 8 * 128, 2 * QB)).astype(F8E4)
        wq_nope = Wq_r[:, heads, :DN].reshape(D_MODEL, HP * DN)
        wq_rope = Wq_r[:, heads, DN:].reshape(D_MODEL, HP * DR)
        wq_c = np.concatenate([wq_nope, wq_rope], axis=1) * (scale * SWQ)
        wq8_c = np.ascontiguousarray(
            wq_c.reshape(8, 2, 128, HP * (DN + DR))
            .transpose(0, 2, 1, 3)
            .reshape(8 * 128, 2 * HP * (DN + DR))).astype(F8E4)
        wkva_c = Wkva_f.astype(BF16)
        wbk_c = np.ascontiguousarray(
            Wb_r[:, heads, :DN].reshape(R, HP * DN)).astype(BF16)
        wbv_c = np.ascontiguousarray(
            Wb_r[:, heads, DN:].reshape(R, HP * DV)).astype(BF16)
        wo_c = np.ascontiguousarray(
            Wo_f[HP * g * DV:(HP * g + HP) * DV, :]).astype(BF16)
        in_maps.append({
            "xT": xTc,
            "xT8": xT8c,
            "wq8": wq8_c,
            "wkva": wkva_c,
            "wkvbk": wbk_c,
            "wkvbv": wbv_c,
            "wo": wo_c,
            "cosf": cosf,
            "sinf": sinf,
            "masks": masks,
            "ident": ident,
            "ones": ones,
            "onesf": onesf,
        })
    return in_maps


def gather_output(results):
    """results: list of 8 dicts with 'outp' block-major bf16 partials."""
    out = np.empty((B, S, D_MODEL), dtype=np.float32)
    for b in range(B):
        acc = results[HP * b]["outp"].astype(np.float32)
        for g in range(1, HP):
            acc += results[HP * b + g]["outp"].astype(np.float32)
        # blocks (m, nb) -> [D_MODEL, S] -> transpose to [S, D_MODEL]
        out[b] = (acc.reshape(16, NQB, 128, QB).transpose(0, 2, 1, 3)
                  .reshape(D_MODEL, S).T)
    return out


def kernel(x, Wq, Wkv_a, Wkv_b, Wo):
    from concourse.bass_utils import run_bass_kernel_spmd

    nc = get_program()
    in_maps = make_core_inputs(x, Wq, Wkv_a, Wkv_b, Wo)
    res = run_bass_kernel_spmd(nc, in_maps, list(range(NCORES)))
    return gather_output(res.results)


# revision 14
# speedup vs baseline: 1.0999x; 1.0999x over previous
"""Multi-Head Latent Attention (MLA) TRN2 Bass kernel.

Sharding: data-parallel over batch (B=2) x tensor-parallel over heads
(16 heads -> 4 per core) = 8 cores. The kv_lora latent path and shared
rope key are computed replicated within each batch group (cross-core
AllGather / Pool-engine offload both trip the chip power throttle and
net out slower -- measured); the final output projection is computed as
per-core partials which the host sums.

Structure (two phases, internally pipelined; fully merging the phases
was measured SLOWER -- cross-engine SBUF/PSUM contention inflates every
matmul by 5-20%):
  Phase 1, per 512-column quarter t: kv_a (bf16) -> q-proj (fp8
  DoubleRow; x and Wq pre-scaled to fp8 on host) -> kv_b (k_nopeT, v)
  -> RoPE -> fp8 casts -> DoubleRow packing. The rope/cast/pack engine
  work of quarter t overlaps quarter t+1's PE matmuls, and attention
  can start right after the last quarter (its first q-block only needs
  quarter-0 data).
  Phase 2: attention units iterated q-block-major ((h, j) with j outer)
  with the output projection of column block j interleaved after the
  second unit of block j+1 -- out-proj is PE-heavy/ACT-light, so it
  fills the PE gaps left by the ACT-bound exp pipeline, and the output
  DMA spreads across the phase instead of draining at the end.

Scores per (head, q-block) via one fp8e4 DoubleRow matmul per k-chunk
(the 192-dim nope+rope contraction packed as 2x96 partitions, q/k
pre-scaled by 64/32 and descaled inside the exp). Chunk PAIRS share one
2-bank PSUM tile; the second chunk's columns are SHIFTED to start at
column 512 of the tile regardless of causal trim, so the pair's exp
input is one contiguous gap-free span (one ACT op, no stale bytes,
minimum width). Causal masks are added on diagonal chunks by a bf16 PE
matmul; exp on ACT with no max subtraction (scores are bounded);
softmax denominators via running elementwise bf16 sums of the exp'd
chunks on the DVE + one ones-matmul per unit, deferred one unit so the
PE never stalls on them; outT accumulated in PSUM and normalized by
broadcasted reciprocals.

x and outp use chunk-major DRAM layouts so every DMA is one dense
contiguous block (host packs/unpacks); outp is bf16 partials summed in
f32 on the host.
"""

import math
import sys

import numpy as np
import ml_dtypes

try:  # concourse ships in the container; fall back to the repo checkout
    import concourse.bass  # noqa: F401
except ImportError:  # pragma: no cover
    for p in ("/opt/trn_rl_repo", "/root/.axon_site/_ro/trn_rl_repo"):
        if p not in sys.path:
            sys.path.insert(0, p)

# Problem constants (hardcoded; harness calls kernel() standalone).
D_MODEL = 2048
N_HEADS = 16
R = 512          # kv lora rank
DN = 128         # d_nope
DR = 64          # d_rope
DV = 128         # d_v
ROPE_THETA = 10000.0
B = 2
S = 2048
HP = 4           # heads per core
QB = 512         # q block size
NKC = S // 128   # 16 k chunks
NQB = S // QB    # 4 q blocks
NCORES = 8

BF16 = ml_dtypes.bfloat16
F8E4 = ml_dtypes.float8_e4m3  # mybir float8e4 (IEEE e4m3, max finite 240)

# fp8 pre-scales. QS/KS: q/k tiles feeding the score matmuls (descaled
# inside the exp). SXQ/SWQ: host-side scales for x / Wq feeding the fp8
# DoubleRow q-projection (descaled in the PSUM->SBUF store).
QS, KS = 64.0, 32.0
SXQ, SWQ = 16.0, 8192.0

_PROGRAM = {}


def _build_program(split_waits=True):
    import concourse.bass as bass
    import concourse.mybir as mybir
    from concourse.tile import TileContext

    def split_multi_waits(max_waits=1):
        """The walrus build in this container rejects instructions with
        more than `max_waits` sync-wait commands. Move excess waits onto
        same-engine NoOps inserted just before the instruction."""
        for f in nc.m.functions:
            for bb in f.blocks:
                out = []
                changed = False
                for inst in bb.instructions:
                    si = getattr(inst, "sync_info", None)
                    ws = list(si.on_wait) if si is not None else []
                    if len(ws) > max_waits:
                        changed = True
                        inst.sync_info = mybir.SyncInfo(
                            on_wait=ws[:max_waits],
                            on_update=list(si.on_update))
                        for w in ws[max_waits:]:
                            n = mybir.InstNoOp(
                                name=nc.get_next_instruction_name(),
                                ins=[], outs=[])
                            n.engine = inst.engine
                            n.sync_info = mybir.SyncInfo(
                                on_wait=[w], on_update=[])
                            out.append(n)
                    out.append(inst)
                if changed:
                    bb.instructions = out

    f32 = mybir.dt.float32
    cdt = mybir.dt.bfloat16
    f8 = mybir.dt.float8e4
    DRmode = mybir.MatmulPerfMode.DoubleRow
    Copy = mybir.ActivationFunctionType.Copy
    Exp = mybir.ActivationFunctionType.Exp
    Ln = mybir.ActivationFunctionType.Ln

    nc = bass.Bass()

    # x arrives chunk-major: block (t, k) = xT_logical[128k:128k+128,
    # 512t:512t+512] stored contiguously so every chunk DMA is one dense
    # 128KB transfer instead of 128 separate 1KB rows.
    xT = nc.dram_tensor("xT", [NQB * 16 * 128, QB], cdt, kind="ExternalInput")
    # fp8 copy of x for the q-projection, pair-major for DoubleRow:
    # block (t, p) = [128, 2, 512]: elem (kp, j, col) =
    # SXQ * x[512t+col, 256p+128j+kp] (feature on partitions).
    xT8 = nc.dram_tensor("xT8", [NQB * 8 * 128, 2 * QB], f8,
                         kind="ExternalInput")
    # Wq in fp8, pair-major: block p = [128, 2, 768]: elem (kp, j, m) =
    # SWQ * scale * Wq[256p+128j+kp, m] (m = head-major nope|rope cols).
    wq8 = nc.dram_tensor("wq8", [8 * 128, 2 * HP * (DN + DR)], f8,
                         kind="ExternalInput")
    wkva = nc.dram_tensor("wkva", [D_MODEL, R + DR], cdt, kind="ExternalInput")
    wkvbk = nc.dram_tensor("wkvbk", [R, HP * DN], cdt, kind="ExternalInput")
    wkvbv = nc.dram_tensor("wkvbv", [R, HP * DV], cdt, kind="ExternalInput")
    wo = nc.dram_tensor("wo", [HP * DV, D_MODEL], cdt, kind="ExternalInput")
    cosf = nc.dram_tensor("cosf", [128, S], cdt, kind="ExternalInput")
    sinf = nc.dram_tensor("sinf", [128, S], cdt, kind="ExternalInput")
    masks = nc.dram_tensor("masks", [128, 128], cdt, kind="ExternalInput")
    ident = nc.dram_tensor("ident", [128, 128], cdt, kind="ExternalInput")
    ones = nc.dram_tensor("ones", [128, 1], cdt, kind="ExternalInput")
    onesf = nc.dram_tensor("onesf", [1, 128], cdt, kind="ExternalInput")
    # outp is block-major: block (m, nb) stored contiguously; host unpacks.
    outp = nc.dram_tensor("outp", [16 * NQB * 128, QB], cdt,
                          kind="ExternalOutput")

    with TileContext(nc) as tc:
        with (
            tc.tile_pool(name="const", bufs=1) as cpool,
            tc.tile_pool(name="persist", bufs=1) as ppool,
        ):
            cosf_sb = cpool.tile([128, S], cdt, name="cosf_sb")
            sinf_sb = cpool.tile([128, S], cdt, name="sinf_sb")
            masks_sb = cpool.tile([128, 128], cdt, name="masks_sb")
            ident_sb = cpool.tile([128, 128], cdt, name="ident_sb")
            ones_sb = cpool.tile([128, 1], cdt, name="ones_sb")
            onesb_sb = cpool.tile([1, 128], cdt, name="onesb_sb")

            # Persistent activations. q_nope / k_nope tiles live in fp8
            # (written pre-scaled straight from PSUM); rope halves stay
            # bf16 until after the RoPE rotation, then are cast.
            qT8 = [
                ppool.tile([128, S], f8, name=f"qT8_{m}", tag="qT8", bufs=4)
                for m in range(4)
            ]
            qTr = [
                ppool.tile([128, S], cdt, name=f"qTr{m}", tag="qT", bufs=2)
                for m in range(2)
            ]
            kn8 = [
                ppool.tile([128, S], f8, name=f"kn8_{m}", tag="kn8", bufs=4)
                for m in range(4)
            ]
            qr8 = [
                ppool.tile([128, S], f8, name=f"qr8_{m}", tag="qr8", bufs=2)
                for m in range(2)
            ]
            kr8 = ppool.tile([128, S], f8, name="kr8", tag="kr8", bufs=1)
            # DoubleRow-packed per-head tensors: 192 contraction dims
            # as 2 blocks of 96 partitions (blk0 = dims 0:96,
            # blk1 = dims 96:192 = nope 96:128 + rope 0:64).
            qf8 = [
                ppool.tile([96, NQB, 2, QB], f8, name=f"qf8_{h}", tag="qf8",
                           bufs=4)
                for h in range(HP)
            ]
            kf8 = [
                ppool.tile([96, NKC, 2, 128], f8, name=f"kf8_{h}", tag="kf8",
                           bufs=4)
                for h in range(HP)
            ]
            ck = [
                ppool.tile([128, S], cdt, name=f"ck{m}", tag="cko", bufs=4)
                for m in range(4)
            ]
            kr = ppool.tile([128, S], cdt, name="krope", tag="krope", bufs=1)
            vt = [
                ppool.tile([128, HP * DV], cdt, name=f"v{i}", tag="v",
                           bufs=NKC)
                for i in range(NKC)
            ]
            def store_q(m, cols, ps):
                # PSUM -> SBUF store for qT M-tile m, descaling the host
                # fp8 pre-scales (SXQ*SWQ); nope tiles also pick up the
                # QS score pre-scale and go straight to fp8.
                if m < 4:
                    nc.scalar.activation(qT8[m][:, cols], ps, Copy,
                                         scale=QS / (SXQ * SWQ))
                else:
                    nc.scalar.activation(qTr[m - 4][:, cols], ps, Copy,
                                         scale=1.0 / (SXQ * SWQ))

            # kv_b weights persist so their DMAs can issue at startup.
            wbk_sb = [
                ppool.tile([128, HP * DN], cdt, name=f"wbk_sb{r}", tag="wbk",
                           bufs=4)
                for r in range(4)
            ]
            wbv_sb = [
                ppool.tile([128, HP * DV], cdt, name=f"wbv_sb{r}", tag="wbv",
                           bufs=4)
                for r in range(4)
            ]

            # ---- Phase 1: per-quarter projections + rope + packing ----
            with (
                tc.tile_pool(name="wproj", bufs=1) as wpool,
                tc.tile_pool(name="xstream", bufs=1) as xpool,
                tc.tile_pool(name="psA", bufs=8, space="PSUM") as psA,
            ):
                # Weight DMAs on the scalar queue; wkva first (the very
                # first PE matmul needs wkva_sb[0]), wq8 interleaved.
                wkva_sb = []
                wq8_sb = []
                for k in range(16):
                    w2 = wpool.tile([128, R + DR], cdt, name=f"wkva_sb{k}",
                                    tag="wkva", bufs=16)
                    nc.scalar.dma_start(w2, wkva[k * 128:(k + 1) * 128, :])
                    wkva_sb.append(w2)
                    if k < 8:
                        w1 = wpool.tile([128, 2, HP * (DN + DR)], f8,
                                        name=f"wq8_sb{k}", tag="wq8", bufs=8)
                        nc.scalar.dma_start(w1, wq8[k * 128:(k + 1) * 128, :])
                        wq8_sb.append(w1)
                for r in range(4):
                    nc.scalar.dma_start(wbk_sb[r],
                                        wkvbk[r * 128:(r + 1) * 128, :])
                    nc.scalar.dma_start(wbv_sb[r],
                                        wkvbv[r * 128:(r + 1) * 128, :])
                nc.scalar.dma_start(cosf_sb, cosf[:, :])
                nc.scalar.dma_start(sinf_sb, sinf[:, :])
                nc.scalar.dma_start(masks_sb, masks[:, :])
                nc.scalar.dma_start(ident_sb, ident[:, :])
                nc.scalar.dma_start(ones_sb, ones[:, :])
                nc.scalar.dma_start(onesb_sb, onesf[:, :])

                for t in range(NQB):
                    tcols = slice(t * QB, (t + 1) * QB)
                    xq = []
                    xq8t = []
                    for k in range(16):
                        xk = xpool.tile([128, QB], cdt, name=f"xq_{t}_{k}",
                                        tag="xq", bufs=24)
                        nc.sync.dma_start(
                            xk,
                            xT[(t * 16 + k) * 128:(t * 16 + k + 1) * 128, :])
                        xq.append(xk)
                        if k % 2 == 1:
                            p = k // 2
                            x8 = xpool.tile([128, 2, QB], f8,
                                            name=f"xq8_{t}_{p}", tag="xq8",
                                            bufs=16)
                            nc.sync.dma_start(
                                x8,
                                xT8[(t * 8 + p) * 128:(t * 8 + p + 1) * 128,
                                    :])
                            xq8t.append(x8)
                    # kv_a first (bf16, DMA-latency friendly at t=0): each
                    # landing x chunk feeds 5 matmuls.
                    ps_k = [
                        psA.tile([128, QB], f32, name=f"psk_{t}_{m}",
                                 tag="ps")
                        for m in range(4)
                    ]
                    ps_r = psA.tile([64, QB], f32, name=f"psr_{t}", tag="ps")
                    for k in range(16):
                        for m in range(4):
                            nc.tensor.matmul(
                                ps_k[m],
                                lhsT=wkva_sb[k][:, m * 128:(m + 1) * 128],
                                rhs=xq[k], start=(k == 0), stop=(k == 15))
                        nc.tensor.matmul(
                            ps_r, lhsT=wkva_sb[k][:, R:R + DR], rhs=xq[k],
                            start=(k == 0), stop=(k == 15))
                    for m in range(4):
                        nc.vector.tensor_copy(ck[m][:, tcols], ps_k[m])
                    nc.scalar.copy(kr[0:64, tcols], ps_r)
                    nc.scalar.copy(kr[64:128, tcols], ps_r)
                    # q projection: fp8 DoubleRow, 2 chunks per matmul.
                    ps_q = [
                        psA.tile([128, QB], f32, name=f"psq_{t}_{m}",
                                 tag="ps")
                        for m in range(6)
                    ]
                    for p in range(8):
                        for m in range(6):
                            nc.tensor.matmul(
                                ps_q[m],
                                lhsT=wq8_sb[p][:, :, m * 128:(m + 1) * 128],
                                rhs=xq8t[p], start=(p == 0), stop=(p == 7),
                                perf_mode=DRmode)
                    for m in range(6):
                        store_q(m, tcols, ps_q[m])
                    # kv up-projection for this quarter: k_nopeT + v.
                    for m in range(4):
                        ps = psA.tile([128, QB], f32, name=f"psn_{t}_{m}",
                                      tag="ps")
                        for r in range(4):
                            nc.tensor.matmul(
                                ps, lhsT=wbk_sb[r][:, m * 128:(m + 1) * 128],
                                rhs=ck[r][:, tcols], start=(r == 0),
                                stop=(r == 3))
                        nc.scalar.activation(kn8[m][:, tcols], ps, Copy,
                                             scale=KS)
                    for ci in range(4):
                        c = 4 * t + ci
                        ps = psA.tile([128, HP * DV], f32, name=f"psv_{c}",
                                      tag="ps")
                        for r in range(4):
                            nc.tensor.matmul(
                                ps, lhsT=ck[r][:, c * 128:(c + 1) * 128],
                                rhs=wbv_sb[r], start=(r == 0), stop=(r == 3))
                        nc.vector.tensor_copy(vt[c], ps)
                    # RoPE for this quarter (DVE + swap DMAs), fp8 casts,
                    # and DoubleRow packing -- all overlap quarter t+1's
                    # PE matmuls.
                    for idx, tapt in enumerate([qTr[0], qTr[1], kr]):
                        sw = ppool.tile([128, QB], cdt, name=f"sw_{t}_{idx}",
                                        tag="sw", bufs=3)
                        for blk in range(4):
                            src = (blk ^ 1) * 32
                            nc.sync.dma_start(
                                sw[blk * 32:(blk + 1) * 32, :],
                                tapt[src:src + 32, tcols])
                        tap = tapt[:, tcols]
                        nc.vector.tensor_mul(tap, tap, cosf_sb[:, tcols])
                        nc.vector.tensor_mul(sw, sw, sinf_sb[:, tcols])
                        nc.vector.tensor_add(tap, tap, sw)
                    nc.scalar.activation(qr8[0][:, tcols], qTr[0][:, tcols],
                                         Copy, scale=QS)
                    nc.scalar.activation(qr8[1][:, tcols], qTr[1][:, tcols],
                                         Copy, scale=QS)
                    nc.scalar.activation(kr8[:, tcols], kr[:, tcols],
                                         Copy, scale=KS)
                    c4 = slice(4 * t, 4 * t + 4)
                    for h in range(HP):
                        off = (h % 2) * 64
                        ri = h // 2
                        nc.sync.dma_start(qf8[h][0:96, t, 0, :],
                                          qT8[h][0:96, tcols])
                        nc.sync.dma_start(qf8[h][0:32, t, 1, :],
                                          qT8[h][96:128, tcols])
                        nc.sync.dma_start(qf8[h][32:96, t, 1, :],
                                          qr8[ri][off:off + 64, tcols])
                        nc.scalar.dma_start(kf8[h][0:96, c4, 0, :],
                                            kn8[h][0:96, tcols])
                        nc.scalar.dma_start(kf8[h][0:32, c4, 1, :],
                                            kn8[h][96:128, tcols])
                        nc.scalar.dma_start(kf8[h][32:96, c4, 1, :],
                                            kr8[off:off + 64, tcols])

            # outT tiles reuse the c_kvT slots (same tag, 4 bufs); ck is
            # fully consumed by the per-quarter kv_b above.
            outT = [
                ppool.tile([128, S], cdt, name=f"outT{h}", tag="cko", bufs=4)
                for h in range(HP)
            ]

            # ---- Phase 2: attention with interleaved output projection ----
            with (
                tc.tile_pool(name="att", bufs=1) as apool,
                tc.tile_pool(name="psS", bufs=4, space="PSUM") as psS,
                tc.tile_pool(name="psO", bufs=2, space="PSUM") as psO,
                tc.tile_pool(name="psD", bufs=1, space="PSUM") as psD,
                tc.tile_pool(name="psBC", bufs=1, space="PSUM") as psBC,
            ):
                wo_sb = [
                    apool.tile([128, D_MODEL], cdt, name=f"wo_sb{r}",
                               tag="wo", bufs=4)
                    for r in range(4)
                ]
                for r in range(4):
                    nc.sync.dma_start(wo_sb[r], wo[r * 128:(r + 1) * 128, :])

                def calc_dps(h, j, ptacc):
                    # Single partition-sum matmul over the accumulated
                    # exp'd chunks (deferred off the critical path).
                    dps = psD.tile([1, QB], f32, name=f"dps_{h}_{j}",
                                   tag="d")
                    nc.tensor.matmul(dps, lhsT=ones_sb, rhs=ptacc,
                                     start=True, stop=True)
                    return dps

                def norm_early(h, j, dps):
                    # 1/denom as exp(-ln(d)) on the ACT engine.
                    rec = apool.tile([1, QB], f32, name=f"rec_{h}_{j}",
                                     tag="rec", bufs=2)
                    nc.scalar.activation(rec, dps, Ln)
                    recb = apool.tile([1, QB], cdt, name=f"recb_{h}_{j}",
                                      tag="recb", bufs=2)
                    nc.scalar.activation(recb, rec, Exp, scale=-1.0)
                    return recb

                def norm_late(h, j, ops, recb):
                    # Broadcast 1/denom across partitions via a K=1
                    # matmul, then scale the out accumulator into outT.
                    qs = slice(j * QB, (j + 1) * QB)
                    bps = psBC.tile([128, QB], f32, name=f"bps_{h}_{j}",
                                    tag="b")
                    nc.tensor.matmul(bps, lhsT=onesb_sb, rhs=recb,
                                     start=True, stop=True)
                    bc = apool.tile([128, QB], f32, name=f"bc_{h}_{j}",
                                    tag="bc", bufs=2)
                    nc.scalar.copy(bc, bps)
                    nc.vector.tensor_mul(outT[h][:, qs], ops, bc)

                def emit_unit(h, j, pend):
                    """One (head, q-block) attention unit with the
                    deferred-normalization pipeline (previous unit's
                    denominator at pair 0, normalize at pair 1)."""
                    ops = psO.tile([128, QB], f32, name=f"ops_{h}_{j}",
                                   tag="o")
                    ptacc = apool.tile([128, QB], cdt, name=f"pta_{h}_{j}",
                                       tag="pta", bufs=2)
                    nch = 4 * (j + 1)
                    for p in range(nch // 2):
                        cA, cB = 2 * p, 2 * p + 1
                        rA, rB = cA - 4 * j, cB - 4 * j
                        colA = max(0, rA * 128)
                        colB = max(0, rB * 128)
                        bw = QB - colB  # B half is shifted to start at QB
                        pairps = psS.tile([128, 2 * QB], f32,
                                          name=f"sps_{h}_{j}_{p}", tag="s",
                                          bufs=2)
                        nc.tensor.matmul(
                            pairps[:, colA:QB],
                            lhsT=kf8[h][:, cA, :, :],
                            rhs=qf8[h][:, j, :, colA:QB],
                            start=True, stop=(rA < 0),
                            perf_mode=DRmode, skip_group_check=True)
                        if rA >= 0:
                            nc.tensor.matmul(
                                pairps[:, colA:colA + 128],
                                lhsT=ident_sb, rhs=masks_sb,
                                start=False, stop=True,
                                skip_group_check=True)
                        nc.tensor.matmul(
                            pairps[:, QB:QB + bw],
                            lhsT=kf8[h][:, cB, :, :],
                            rhs=qf8[h][:, j, :, colB:QB],
                            start=True, stop=(rB < 0),
                            perf_mode=DRmode, skip_group_check=True)
                        if rB >= 0:
                            nc.tensor.matmul(
                                pairps[:, QB:QB + 128],
                                lhsT=ident_sb, rhs=masks_sb,
                                start=False, stop=True,
                                skip_group_check=True)
                        # One contiguous exp over both halves.
                        ptp = apool.tile([128, 2 * QB], cdt,
                                         name=f"pt_{h}_{j}_{p}", tag="pt",
                                         bufs=2)
                        nc.scalar.activation(
                            ptp[:, colA:QB + bw], pairps[:, colA:QB + bw],
                            Exp, scale=1.0 / (QS * KS))
                        nc.tensor.matmul(
                            ops[:, colA:QB],
                            lhsT=vt[cA][:, h * DV:(h + 1) * DV],
                            rhs=ptp[:, colA:QB], start=(cA == 0),
                            stop=False, skip_group_check=True)
                        nc.tensor.matmul(
                            ops[:, colB:QB],
                            lhsT=vt[cB][:, h * DV:(h + 1) * DV],
                            rhs=ptp[:, QB:QB + bw],
                            start=False, stop=(cB == nch - 1),
                            skip_group_check=True)
                        if cA == 0:
                            nc.vector.tensor_copy(ptacc, ptp[:, 0:QB])
                        else:
                            nc.vector.tensor_add(
                                ptacc[:, colA:QB], ptacc[:, colA:QB],
                                ptp[:, colA:QB])
                        nc.vector.tensor_add(
                            ptacc[:, colB:QB], ptacc[:, colB:QB],
                            ptp[:, QB:QB + bw])
                        if p == 0 and pend is not None and len(pend) == 4:
                            ph, pj, pops, pacc = pend
                            recb = norm_early(ph, pj,
                                              calc_dps(ph, pj, pacc))
                            pend = (ph, pj, pops, recb, True)
                        if p == 1 and pend is not None and len(pend) == 5:
                            ph, pj, pops, recb, _ = pend
                            norm_late(ph, pj, pops, recb)
                            pend = None
                    return (h, j, ops, ptacc)

                def emit_outproj(nb):
                    """Output projection for column block nb: 16 M-tiles
                    as 8 s-ring pair tiles, each block staged and DMA'd
                    out as soon as its copy lands."""
                    ncols = slice(nb * QB, (nb + 1) * QB)
                    for mp in range(8):
                        tile = psS.tile([128, 2 * QB], f32,
                                        name=f"pw_{nb}_{mp}", tag="s",
                                        bufs=2)
                        for i in range(2):
                            m = 2 * mp + i
                            reg = tile[:, i * QB:(i + 1) * QB]
                            for r in range(4):
                                nc.tensor.matmul(
                                    reg,
                                    lhsT=wo_sb[r][:, m * 128:(m + 1) * 128],
                                    rhs=outT[r][:, ncols], start=(r == 0),
                                    stop=(r == 3), skip_group_check=True)
                            st = apool.tile([128, QB], cdt,
                                            name=f"st_{nb}_{m}", tag="st",
                                            bufs=4)
                            if i == 0:
                                nc.scalar.copy(st, reg)
                            else:
                                nc.vector.tensor_copy(st, reg)
                            eng = nc.sync if i == 0 else nc.scalar
                            eng.dma_start(
                                outp[(m * NQB + nb) * 128:
                                     (m * NQB + nb + 1) * 128, :], st)

                pend = None
                pending_out = None
                for j in range(NQB):
                    for h in range(HP):
                        pend = emit_unit(h, j, pend)
                        if pending_out is not None and h == 1:
                            # Column block j-1 is fully normalized by now
                            # (its last unit's norm ran during unit 0).
                            emit_outproj(pending_out)
                            pending_out = None
                    pending_out = j
                # Final flush: last unit's norm + last column block.
                ph, pj, pops, pacc = pend
                recb = norm_early(ph, pj, calc_dps(ph, pj, pacc))
                norm_late(ph, pj, pops, recb)
                emit_outproj(NQB - 1)

    if split_waits:
        split_multi_waits()
    return nc


def get_program(split_waits=True):
    key = (split_waits,)
    if key not in _PROGRAM:
        _PROGRAM[key] = _build_program(split_waits)
    return _PROGRAM[key]


def make_core_inputs(x, Wq, Wkv_a, Wkv_b, Wo):
    """Host-side sharding/pre-processing. Returns list of 8 input dicts."""
    scale = 1.0 / math.sqrt(DN + DR)

    inv_freq = 1.0 / (ROPE_THETA ** (np.arange(0, DR, 2, dtype=np.float64) / DR))
    t = np.arange(S, dtype=np.float64)
    freqs = np.outer(t, inv_freq)                      # [S, 32]
    cos32 = np.cos(freqs).T.astype(np.float32)         # [32, S]
    sin32 = np.sin(freqs).T.astype(np.float32)
    cosf = np.tile(cos32, (4, 1)).astype(BF16)         # [128, S]
    sinf = np.tile(np.concatenate([-sin32, sin32], axis=0), (2, 1)).astype(BF16)

    row = np.arange(128)[:, None]
    col = np.arange(128)[None, :]
    masks = np.where(col >= row, 0.0, -1e30).astype(BF16)  # [128, 128]
    ident = np.eye(128, dtype=BF16)
    ones = np.ones([128, 1], dtype=BF16)
    onesf = np.ones([1, 128], dtype=BF16)

    Wq_r = np.asarray(Wq, dtype=np.float32).reshape(D_MODEL, N_HEADS, DN + DR)
    Wb_r = np.asarray(Wkv_b, dtype=np.float32).reshape(R, N_HEADS, DN + DV)
    Wo_f = np.asarray(Wo, dtype=np.float32)
    Wkva_f = np.asarray(Wkv_a, dtype=np.float32)
    x_f = np.asarray(x, dtype=np.float32)

    in_maps = []
    for c in range(NCORES):
        b, g = divmod(c, HP)
        heads = list(range(HP * g, HP * g + HP))
        # chunk-major xT: block (t, k) contiguous [128, 512]
        xTc = np.ascontiguousarray(
            x_f[b].T.reshape(16, 128, NQB, QB).transpose(2, 0, 1, 3)
            .reshape(NQB * 16 * 128, QB)).astype(BF16)
        # fp8 pair-major x for the DoubleRow q projection.
        xT8c = np.ascontiguousarray(
            (x_f[b].T * SXQ).reshape(8, 2, 128, NQB, QB)
            .transpose(3, 0, 2, 1, 4)
            .reshape(NQB * 8 * 128, 2 * QB)).astype(F8E4)
        wq_nope = Wq_r[:, heads, :DN].reshape(D_MODEL, HP * DN)
        wq_rope = Wq_r[:, heads, DN:].reshape(D_MODEL, HP * DR)
        wq_c = np.concatenate([wq_nope, wq_rope], axis=1) * (scale * SWQ)
        wq8_c = np.ascontiguousarray(
            wq_c.reshape(8, 2, 128, HP * (DN + DR))
            .transpose(0, 2, 1, 3)
            .reshape(8 * 128, 2 * HP * (DN + DR))).astype(F8E4)
        wkva_c = Wkva_f.astype(BF16)
        wbk_c = np.ascontiguousarray(
            Wb_r[:, heads, :DN].reshape(R, HP * DN)).astype(BF16)
        wbv_c = np.ascontiguousarray(
            Wb_r[:, heads, DN:].reshape(R, HP * DV)).astype(BF16)
        wo_c = np.ascontiguousarray(
            Wo_f[HP * g * DV:(HP * g + HP) * DV, :]).astype(BF16)
        in_maps.append({
            "xT": xTc,
            "xT8": xT8c,
            "wq8": wq8_c,
            "wkva": wkva_c,
            "wkvbk": wbk_c,
            "wkvbv": wbv_c,
            "wo": wo_c,
            "cosf": cosf,
            "sinf": sinf,
            "masks": masks,
            "ident": ident,
            "ones": ones,
            "onesf": onesf,
        })
    return in_maps


def gather_output(results):
    """results: list of 8 dicts with 'outp' block-major bf16 partials."""
    out = np.empty((B, S, D_MODEL), dtype=np.float32)
    for b in range(B):
        acc = results[HP * b]["outp"].astype(np.float32)
        for g in range(1, HP):
            acc += results[HP * b + g]["outp"].astype(np.float32)
        # blocks (m, nb) -> [D_MODEL, S] -> transpose to [S, D_MODEL]
        out[b] = (acc.reshape(16, NQB, 128, QB).transpose(0, 2, 1, 3)
                  .reshape(D_MODEL, S).T)
    return out


def kernel(x, Wq, Wkv_a, Wkv_b, Wo):
    from concourse.bass_utils import run_bass_kernel_spmd

    nc = get_program()
    in_maps = make_core_inputs(x, Wq, Wkv_a, Wkv_b, Wo)
    res = run_bass_kernel_spmd(nc, in_maps, list(range(NCORES)))
    return gather_output(res.results)


# revision 21
# speedup vs baseline: 1.1060x; 1.0056x over previous
"""Multi-Head Latent Attention (MLA) TRN2 Bass kernel.

Sharding: data-parallel over batch (B=2) x tensor-parallel over heads
(16 heads -> 4 per core) = 8 cores. The kv_lora latent path and shared
rope key are computed replicated within each batch group (cross-core
AllGather / Pool-engine offload both trip the chip power throttle and
net out slower -- measured); the final output projection is computed as
per-core partials which the host sums.

Structure (two phases, internally pipelined; fully merging the phases
was measured SLOWER -- cross-engine SBUF/PSUM contention inflates every
matmul by 5-20%):
  Phase 1, per 512-column quarter t: kv_a (bf16) -> q-proj (fp8
  DoubleRow; x and Wq pre-scaled to fp8 on host) -> kv_b (k_nopeT, v)
  -> RoPE -> fp8 casts -> DoubleRow packing. The rope/cast/pack engine
  work of quarter t overlaps quarter t+1's PE matmuls, and attention
  can start right after the last quarter (its first q-block only needs
  quarter-0 data).
  Phase 2: attention units iterated q-block-major ((h, j) with j outer)
  with the output projection of column block j interleaved after the
  second unit of block j+1 -- out-proj is PE-heavy/ACT-light, so it
  fills the PE gaps left by the ACT-bound exp pipeline, and the output
  DMA spreads across the phase instead of draining at the end.

Scores per (head, q-block) via one fp8e4 DoubleRow matmul per k-chunk
(the 192-dim nope+rope contraction packed as 2x96 partitions, q/k
pre-scaled by 64/32 and descaled inside the exp). Chunk PAIRS share one
2-bank PSUM tile; the second chunk's columns are SHIFTED to start at
column 512 of the tile regardless of causal trim, so the pair's exp
input is one contiguous gap-free span (one ACT op, no stale bytes,
minimum width). Causal masks are added on diagonal chunks by a bf16 PE
matmul; exp on ACT with no max subtraction (scores are bounded);
softmax denominators via running elementwise bf16 sums of the exp'd
chunks on the DVE + one ones-matmul per unit, deferred one unit so the
PE never stalls on them; outT accumulated in PSUM and normalized by
broadcasted reciprocals.

x and outp use chunk-major DRAM layouts so every DMA is one dense
contiguous block (host packs/unpacks); outp is bf16 partials summed in
f32 on the host.
"""

import math
import sys

import numpy as np
import ml_dtypes

try:  # concourse ships in the container; fall back to the repo checkout
    import concourse.bass  # noqa: F401
except ImportError:  # pragma: no cover
    for p in ("/opt/trn_rl_repo", "/root/.axon_site/_ro/trn_rl_repo"):
        if p not in sys.path:
            sys.path.insert(0, p)

# Problem constants (hardcoded; harness calls kernel() standalone).
D_MODEL = 2048
N_HEADS = 16
R = 512          # kv lora rank
DN = 128         # d_nope
DR = 64          # d_rope
DV = 128         # d_v
ROPE_THETA = 10000.0
B = 2
S = 2048
HP = 4           # heads per core
QB = 512         # q block size
NKC = S // 128   # 16 k chunks
NQB = S // QB    # 4 q blocks
NCORES = 8

BF16 = ml_dtypes.bfloat16
F8E4 = ml_dtypes.float8_e4m3  # mybir float8e4 (IEEE e4m3, max finite 240)

# fp8 pre-scales. QS/KS: q/k tiles feeding the score matmuls (descaled
# inside the exp). SXQ/SWQ: host-side scales for x / Wq feeding the fp8
# DoubleRow q-projection (descaled in the PSUM->SBUF store).
QS, KS = 64.0, 32.0
SXQ, SWQ = 16.0, 8192.0

_PROGRAM = {}


def _build_program(split_waits=True):
    import concourse.bass as bass
    import concourse.bass_isa as bass_isa
    import concourse.mybir as mybir
    from concourse.tile import TileContext

    def split_multi_waits(max_waits=1):
        """The walrus build in this container rejects instructions with
        more than `max_waits` sync-wait commands. Move excess waits onto
        same-engine NoOps inserted just before the instruction."""
        for f in nc.m.functions:
            for bb in f.blocks:
                out = []
                changed = False
                for inst in bb.instructions:
                    si = getattr(inst, "sync_info", None)
                    ws = list(si.on_wait) if si is not None else []
                    is_pool = getattr(inst, "engine", None) == \
                        mybir.EngineType.Pool
                    if len(ws) > max_waits and not is_pool:
                        changed = True
                        inst.sync_info = mybir.SyncInfo(
                            on_wait=ws[:max_waits],
                            on_update=list(si.on_update))
                        for w in ws[max_waits:]:
                            n = mybir.InstNoOp(
                                name=nc.get_next_instruction_name(),
                                ins=[], outs=[])
                            n.engine = inst.engine
                            n.sync_info = mybir.SyncInfo(
                                on_wait=[w], on_update=[])
                            out.append(n)
                    out.append(inst)
                if changed:
                    bb.instructions = out

    f32 = mybir.dt.float32
    cdt = mybir.dt.bfloat16
    f8 = mybir.dt.float8e4
    DRmode = mybir.MatmulPerfMode.DoubleRow
    Copy = mybir.ActivationFunctionType.Copy
    Exp = mybir.ActivationFunctionType.Exp
    Ln = mybir.ActivationFunctionType.Ln

    nc = bass.Bass()

    # x arrives chunk-major: block (t, k) = xT_logical[128k:128k+128,
    # 512t:512t+512] stored contiguously so every chunk DMA is one dense
    # 128KB transfer instead of 128 separate 1KB rows.
    xT = nc.dram_tensor("xT", [NQB * 16 * 128, QB], cdt, kind="ExternalInput")
    # fp8 copy of x for the q-projection, pair-major for DoubleRow:
    # block (t, p) = [128, 2, 512]: elem (kp, j, col) =
    # SXQ * x[512t+col, 256p+128j+kp] (feature on partitions).
    xT8 = nc.dram_tensor("xT8", [NQB * 8 * 128, 2 * QB], f8,
                         kind="ExternalInput")
    # Wq in fp8, pair-major: block p = [128, 2, 768]: elem (kp, j, m) =
    # SWQ * scale * Wq[256p+128j+kp, m] (m = head-major nope|rope cols).
    wq8 = nc.dram_tensor("wq8", [8 * 128, 2 * HP * (DN + DR)], f8,
                         kind="ExternalInput")
    # wkva cols: 512 latent | 64 rope | 64 rope again (duplicated so the
    # rope projection runs as one full 128-wide matmul and lands already
    # row-duplicated for the swap-rope layout).
    wkva = nc.dram_tensor("wkva", [D_MODEL, R + 2 * DR], cdt,
                          kind="ExternalInput")
    wkvbk = nc.dram_tensor("wkvbk", [R, HP * DN], cdt, kind="ExternalInput")
    wkvbv = nc.dram_tensor("wkvbv", [R, HP * DV], cdt, kind="ExternalInput")
    wo = nc.dram_tensor("wo", [HP * DV, D_MODEL], cdt, kind="ExternalInput")
    cosf = nc.dram_tensor("cosf", [128, S], cdt, kind="ExternalInput")
    sinf = nc.dram_tensor("sinf", [128, S], cdt, kind="ExternalInput")
    masks = nc.dram_tensor("masks", [128, 128], cdt, kind="ExternalInput")
    ident = nc.dram_tensor("ident", [128, 128], cdt, kind="ExternalInput")
    ones = nc.dram_tensor("ones", [128, 1], cdt, kind="ExternalInput")
    onesf = nc.dram_tensor("onesf", [1, 128], cdt, kind="ExternalInput")
    # outp is block-major: block (m, nb) stored contiguously; host unpacks.
    outp = nc.dram_tensor("outp", [16 * NQB * 128, QB], cdt,
                          kind="ExternalOutput")

    with TileContext(nc) as tc:
        with (
            tc.tile_pool(name="const", bufs=1) as cpool,
            tc.tile_pool(name="persist", bufs=1) as ppool,
        ):
            cosf_sb = cpool.tile([128, S], cdt, name="cosf_sb")
            sinf_sb = cpool.tile([128, S], cdt, name="sinf_sb")
            masks_sb = cpool.tile([128, 128], cdt, name="masks_sb")
            ident_sb = cpool.tile([128, 128], cdt, name="ident_sb")
            ones_sb = cpool.tile([128, 1], cdt, name="ones_sb")
            onesb_sb = cpool.tile([1, 128], cdt, name="onesb_sb")

            # Persistent activations. q_nope / k_nope tiles live in fp8
            # (written pre-scaled straight from PSUM); rope halves stay
            # bf16 until after the RoPE rotation, then are cast.
            qT8 = [
                ppool.tile([128, S], f8, name=f"qT8_{m}", tag="qT8", bufs=4)
                for m in range(4)
            ]
            qTr = [
                ppool.tile([128, S], cdt, name=f"qTr{m}", tag="qT", bufs=2)
                for m in range(2)
            ]
            kn8 = [
                ppool.tile([128, S], f8, name=f"kn8_{m}", tag="kn8", bufs=4)
                for m in range(4)
            ]
            qr8 = [
                ppool.tile([128, S], f8, name=f"qr8_{m}", tag="qr8", bufs=2)
                for m in range(2)
            ]
            kr8 = ppool.tile([128, S], f8, name="kr8", tag="kr8", bufs=1)
            # DoubleRow-packed per-head tensors: 192 contraction dims
            # as 2 blocks of 96 partitions (blk0 = dims 0:96,
            # blk1 = dims 96:192 = nope 96:128 + rope 0:64).
            qf8 = [
                ppool.tile([96, NQB, 2, QB], f8, name=f"qf8_{h}", tag="qf8",
                           bufs=4)
                for h in range(HP)
            ]
            kf8 = [
                ppool.tile([96, NKC, 2, 128], f8, name=f"kf8_{h}", tag="kf8",
                           bufs=4)
                for h in range(HP)
            ]
            ck = [
                ppool.tile([128, S], cdt, name=f"ck{m}", tag="cko", bufs=4)
                for m in range(4)
            ]
            kr = ppool.tile([128, S], cdt, name="krope", tag="krope", bufs=1)
            vt = [
                ppool.tile([128, HP * DV], cdt, name=f"v{i}", tag="v",
                           bufs=NKC)
                for i in range(NKC)
            ]
            def store_q(m, cols, ps):
                # PSUM -> SBUF store for qT M-tile m, descaling the host
                # fp8 pre-scales (SXQ*SWQ); nope tiles also pick up the
                # QS score pre-scale and go straight to fp8.
                if m < 4:
                    nc.scalar.activation(qT8[m][:, cols], ps, Copy,
                                         scale=QS / (SXQ * SWQ))
                else:
                    nc.scalar.activation(qTr[m - 4][:, cols], ps, Copy,
                                         scale=1.0 / (SXQ * SWQ))

            # kv_b weights persist so their DMAs can issue at startup.
            wbk_sb = [
                ppool.tile([128, HP * DN], cdt, name=f"wbk_sb{r}", tag="wbk",
                           bufs=4)
                for r in range(4)
            ]
            wbv_sb = [
                ppool.tile([128, HP * DV], cdt, name=f"wbv_sb{r}", tag="wbv",
                           bufs=4)
                for r in range(4)
            ]

            # ---- Phase 1: per-quarter projections + rope + packing ----
            with (
                tc.tile_pool(name="wproj", bufs=1) as wpool,
                tc.tile_pool(name="xstream", bufs=1) as xpool,
                tc.tile_pool(name="psA", bufs=8, space="PSUM") as psA,
            ):
                # Weight DMAs on the scalar queue; wkva first (the very
                # first PE matmul needs wkva_sb[0]), wq8 interleaved.
                wkva_sb = []
                wq8_sb = []
                for k in range(16):
                    w2 = wpool.tile([128, R + 2 * DR], cdt,
                                    name=f"wkva_sb{k}", tag="wkva", bufs=16)
                    nc.scalar.dma_start(w2, wkva[k * 128:(k + 1) * 128, :])
                    wkva_sb.append(w2)
                    if k < 8:
                        w1 = wpool.tile([128, 2, HP * (DN + DR)], f8,
                                        name=f"wq8_sb{k}", tag="wq8", bufs=8)
                        nc.scalar.dma_start(w1, wq8[k * 128:(k + 1) * 128, :])
                        wq8_sb.append(w1)
                for r in range(4):
                    nc.scalar.dma_start(wbk_sb[r],
                                        wkvbk[r * 128:(r + 1) * 128, :])
                    nc.scalar.dma_start(wbv_sb[r],
                                        wkvbv[r * 128:(r + 1) * 128, :])
                nc.scalar.dma_start(cosf_sb, cosf[:, :])
                nc.scalar.dma_start(sinf_sb, sinf[:, :])
                nc.scalar.dma_start(masks_sb, masks[:, :])
                nc.scalar.dma_start(ident_sb, ident[:, :])
                nc.scalar.dma_start(ones_sb, ones[:, :])
                nc.scalar.dma_start(onesb_sb, onesf[:, :])

                for t in range(NQB):
                    tcols = slice(t * QB, (t + 1) * QB)
                    xq = []
                    xq8t = []
                    for k in range(16):
                        xk = xpool.tile([128, QB], cdt, name=f"xq_{t}_{k}",
                                        tag="xq", bufs=24)
                        nc.sync.dma_start(
                            xk,
                            xT[(t * 16 + k) * 128:(t * 16 + k + 1) * 128, :])
                        xq.append(xk)
                        if k % 2 == 1:
                            p = k // 2
                            x8 = xpool.tile([128, 2, QB], f8,
                                            name=f"xq8_{t}_{p}", tag="xq8",
                                            bufs=16)
                            nc.sync.dma_start(
                                x8,
                                xT8[(t * 8 + p) * 128:(t * 8 + p + 1) * 128,
                                    :])
                            xq8t.append(x8)
                    # kv_a first (bf16, DMA-latency friendly at t=0): each
                    # landing x chunk feeds 5 matmuls.
                    ps_k = [
                        psA.tile([128, QB], f32, name=f"psk_{t}_{m}",
                                 tag="ps")
                        for m in range(4)
                    ]
                    ps_r = psA.tile([128, QB], f32, name=f"psr_{t}",
                                    tag="ps")
                    for k in range(16):
                        for m in range(4):
                            nc.tensor.matmul(
                                ps_k[m],
                                lhsT=wkva_sb[k][:, m * 128:(m + 1) * 128],
                                rhs=xq[k], start=(k == 0), stop=(k == 15))
                        nc.tensor.matmul(
                            ps_r, lhsT=wkva_sb[k][:, R:R + 2 * DR],
                            rhs=xq[k], start=(k == 0), stop=(k == 15))
                    for m in range(4):
                        nc.vector.tensor_copy(ck[m][:, tcols], ps_k[m])
                    nc.scalar.copy(kr[:, tcols], ps_r)
                    # q projection: fp8 DoubleRow, 2 chunks per matmul.
                    ps_q = [
                        psA.tile([128, QB], f32, name=f"psq_{t}_{m}",
                                 tag="ps")
                        for m in range(6)
                    ]
                    for p in range(8):
                        for m in range(6):
                            nc.tensor.matmul(
                                ps_q[m],
                                lhsT=wq8_sb[p][:, :, m * 128:(m + 1) * 128],
                                rhs=xq8t[p], start=(p == 0), stop=(p == 7),
                                perf_mode=DRmode)
                    for m in range(6):
                        store_q(m, tcols, ps_q[m])
                    # kv up-projection for this quarter: k_nopeT + v.
                    for m in range(4):
                        ps = psA.tile([128, QB], f32, name=f"psn_{t}_{m}",
                                      tag="ps")
                        for r in range(4):
                            nc.tensor.matmul(
                                ps, lhsT=wbk_sb[r][:, m * 128:(m + 1) * 128],
                                rhs=ck[r][:, tcols], start=(r == 0),
                                stop=(r == 3))
                        nc.scalar.activation(kn8[m][:, tcols], ps, Copy,
                                             scale=KS)
                    for ci in range(4):
                        c = 4 * t + ci
                        ps = psA.tile([128, HP * DV], f32, name=f"psv_{c}",
                                      tag="ps")
                        for r in range(4):
                            nc.tensor.matmul(
                                ps, lhsT=ck[r][:, c * 128:(c + 1) * 128],
                                rhs=wbv_sb[r], start=(r == 0), stop=(r == 3))
                        nc.vector.tensor_copy(vt[c], ps)
                    # RoPE for this quarter (DVE + swap DMAs), fp8 casts,
                    # and DoubleRow packing -- all overlap quarter t+1's
                    # PE matmuls.
                    for idx, tapt in enumerate([qTr[0], qTr[1], kr]):
                        sw = ppool.tile([128, QB], cdt, name=f"sw_{t}_{idx}",
                                        tag="sw", bufs=3)
                        for blk in range(4):
                            src = (blk ^ 1) * 32
                            nc.sync.dma_start(
                                sw[blk * 32:(blk + 1) * 32, :],
                                tapt[src:src + 32, tcols])
                        tap = tapt[:, tcols]
                        nc.vector.tensor_mul(tap, tap, cosf_sb[:, tcols])
                        nc.vector.tensor_mul(sw, sw, sinf_sb[:, tcols])
                        nc.vector.tensor_add(tap, tap, sw)
                    nc.scalar.activation(qr8[0][:, tcols], qTr[0][:, tcols],
                                         Copy, scale=QS)
                    nc.scalar.activation(qr8[1][:, tcols], qTr[1][:, tcols],
                                         Copy, scale=QS)
                    nc.scalar.activation(kr8[:, tcols], kr[:, tcols],
                                         Copy, scale=KS)
                    c4 = slice(4 * t, 4 * t + 4)
                    for h in range(HP):
                        off = (h % 2) * 64
                        ri = h // 2
                        nc.sync.dma_start(qf8[h][0:96, t, 0, :],
                                          qT8[h][0:96, tcols])
                        nc.sync.dma_start(qf8[h][0:32, t, 1, :],
                                          qT8[h][96:128, tcols])
                        nc.sync.dma_start(qf8[h][32:96, t, 1, :],
                                          qr8[ri][off:off + 64, tcols])
                        nc.scalar.dma_start(kf8[h][0:96, c4, 0, :],
                                            kn8[h][0:96, tcols])
                        nc.scalar.dma_start(kf8[h][0:32, c4, 1, :],
                                            kn8[h][96:128, tcols])
                        nc.scalar.dma_start(kf8[h][32:96, c4, 1, :],
                                            kr8[off:off + 64, tcols])

            # outT tiles reuse the c_kvT slots (same tag, 4 bufs); ck is
            # fully consumed by the per-quarter kv_b above.
            outT = [
                ppool.tile([128, S], cdt, name=f"outT{h}", tag="cko", bufs=4)
                for h in range(HP)
            ]

            # ---- Phase 2: attention with interleaved output projection ----
            with (
                tc.tile_pool(name="att", bufs=1) as apool,
                tc.tile_pool(name="psS", bufs=4, space="PSUM") as psS,
                tc.tile_pool(name="psO", bufs=2, space="PSUM") as psO,
                tc.tile_pool(name="psD", bufs=1, space="PSUM") as psD,
                tc.tile_pool(name="psBC", bufs=1, space="PSUM") as psBC,
            ):
                wo_sb = [
                    apool.tile([128, D_MODEL], cdt, name=f"wo_sb{r}",
                               tag="wo", bufs=4)
                    for r in range(4)
                ]
                for r in range(4):
                    nc.sync.dma_start(wo_sb[r], wo[r * 128:(r + 1) * 128, :])

                def calc_dps(h, j, ptacc):
                    # Single partition-sum matmul over the accumulated
                    # exp'd chunks (deferred off the critical path). The
                    # GPSIMD partition_all_reduce variant is unbuildable
                    # here (this walrus has no Pool custom-ISA lowering).
                    dps = psD.tile([1, QB], f32, name=f"dps_{h}_{j}",
                                   tag="d")
                    nc.tensor.matmul(dps, lhsT=ones_sb, rhs=ptacc,
                                     start=True, stop=True)
                    return dps

                def norm_early(h, j, dps):
                    # 1/denom as exp(-ln(d)) on the ACT engine.
                    rec = apool.tile([1, QB], f32, name=f"rec_{h}_{j}",
                                     tag="rec", bufs=2)
                    nc.scalar.activation(rec, dps, Ln)
                    recb = apool.tile([1, QB], cdt, name=f"recb_{h}_{j}",
                                      tag="recb", bufs=2)
                    nc.scalar.activation(recb, rec, Exp, scale=-1.0)
                    return recb

                def norm_late(h, j, ops, recb):
                    # Broadcast 1/denom across partitions via a K=1
                    # matmul, then scale the out accumulator into outT.
                    qs = slice(j * QB, (j + 1) * QB)
                    bps = psBC.tile([128, QB], f32, name=f"bps_{h}_{j}",
                                    tag="b")
                    nc.tensor.matmul(bps, lhsT=onesb_sb, rhs=recb,
                                     start=True, stop=True)
                    bc = apool.tile([128, QB], f32, name=f"bc_{h}_{j}",
                                    tag="bc", bufs=2)
                    nc.scalar.copy(bc, bps)
                    nc.vector.tensor_mul(outT[h][:, qs], ops, bc)

                def emit_unit(h, j, pend):
                    """One (head, q-block) attention unit with the
                    deferred-normalization pipeline (previous unit's
                    denominator at pair 0, normalize at pair 1)."""
                    ops = psO.tile([128, QB], f32, name=f"ops_{h}_{j}",
                                   tag="o")
                    ptacc = apool.tile([128, QB], cdt, name=f"pta_{h}_{j}",
                                       tag="pta", bufs=2)
                    nch = 4 * (j + 1)
                    for p in range(nch // 2):
                        cA, cB = 2 * p, 2 * p + 1
                        rA, rB = cA - 4 * j, cB - 4 * j
                        colA = max(0, rA * 128)
                        colB = max(0, rB * 128)
                        bw = QB - colB  # B half is shifted to start at QB
                        pairps = psS.tile([128, 2 * QB], f32,
                                          name=f"sps_{h}_{j}_{p}", tag="s",
                                          bufs=2)
                        nc.tensor.matmul(
                            pairps[:, colA:QB],
                            lhsT=kf8[h][:, cA, :, :],
                            rhs=qf8[h][:, j, :, colA:QB],
                            start=True, stop=(rA < 0),
                            perf_mode=DRmode, skip_group_check=True)
                        if rA >= 0:
                            nc.tensor.matmul(
                                pairps[:, colA:colA + 128],
                                lhsT=ident_sb, rhs=masks_sb,
                                start=False, stop=True,
                                skip_group_check=True)
                        nc.tensor.matmul(
                            pairps[:, QB:QB + bw],
                            lhsT=kf8[h][:, cB, :, :],
                            rhs=qf8[h][:, j, :, colB:QB],
                            start=True, stop=(rB < 0),
                            perf_mode=DRmode, skip_group_check=True)
                        if rB >= 0:
                            nc.tensor.matmul(
                                pairps[:, QB:QB + 128],
                                lhsT=ident_sb, rhs=masks_sb,
                                start=False, stop=True,
                                skip_group_check=True)
                        # One contiguous exp over both halves.
                        ptp = apool.tile([128, 2 * QB], cdt,
                                         name=f"pt_{h}_{j}_{p}", tag="pt",
                                         bufs=2)
                        nc.scalar.activation(
                            ptp[:, colA:QB + bw], pairps[:, colA:QB + bw],
                            Exp, scale=1.0 / (QS * KS))
                        nc.tensor.matmul(
                            ops[:, colA:QB],
                            lhsT=vt[cA][:, h * DV:(h + 1) * DV],
                            rhs=ptp[:, colA:QB], start=(cA == 0),
                            stop=False, skip_group_check=True)
                        nc.tensor.matmul(
                            ops[:, colB:QB],
                            lhsT=vt[cB][:, h * DV:(h + 1) * DV],
                            rhs=ptp[:, QB:QB + bw],
                            start=False, stop=(cB == nch - 1),
                            skip_group_check=True)
                        if cA == 0:
                            nc.vector.tensor_copy(ptacc, ptp[:, 0:QB])
                        else:
                            nc.vector.tensor_add(
                                ptacc[:, colA:QB], ptacc[:, colA:QB],
                                ptp[:, colA:QB])
                        nc.vector.tensor_add(
                            ptacc[:, colB:QB], ptacc[:, colB:QB],
                            ptp[:, QB:QB + bw])
                        if p == 0 and pend is not None and len(pend) == 4:
                            ph, pj, pops, pacc = pend
                            recb = norm_early(ph, pj,
                                              calc_dps(ph, pj, pacc))
                            pend = (ph, pj, pops, recb, True)
                        if p == 1 and pend is not None and len(pend) == 5:
                            ph, pj, pops, recb, _ = pend
                            norm_late(ph, pj, pops, recb)
                            pend = None
                    return (h, j, ops, ptacc)

                def emit_outproj(nb):
                    """Output projection for column block nb: 16 M-tiles
                    as 8 s-ring pair tiles, each block staged and DMA'd
                    out as soon as its copy lands."""
                    ncols = slice(nb * QB, (nb + 1) * QB)
                    for mp in range(8):
                        tile = psS.tile([128, 2 * QB], f32,
                                        name=f"pw_{nb}_{mp}", tag="s",
                                        bufs=2)
                        for i in range(2):
                            m = 2 * mp + i
                            reg = tile[:, i * QB:(i + 1) * QB]
                            for r in range(4):
                                nc.tensor.matmul(
                                    reg,
                                    lhsT=wo_sb[r][:, m * 128:(m + 1) * 128],
                                    rhs=outT[r][:, ncols], start=(r == 0),
                                    stop=(r == 3), skip_group_check=True)
                            st = apool.tile([128, QB], cdt,
                                            name=f"st_{nb}_{m}", tag="st",
                                            bufs=4)
                            if i == 0:
                                nc.scalar.copy(st, reg)
                            else:
                                nc.vector.tensor_copy(st, reg)
                            eng = nc.sync if i == 0 else nc.scalar
                            eng.dma_start(
                                outp[(m * NQB + nb) * 128:
                                     (m * NQB + nb + 1) * 128, :], st)

                pend = None
                for j in range(NQB):
                    for h in range(HP):
                        pend = emit_unit(h, j, pend)
                # Final flush, then the output projection as one clean
                # PE-bound block (interleaving it with the ACT-bound
                # attention units was measured slower: cross-engine
                # contention taxes every matmul ~15%).
                ph, pj, pops, pacc = pend
                recb = norm_early(ph, pj, calc_dps(ph, pj, pacc))
                norm_late(ph, pj, pops, recb)
                for nb in range(NQB):
                    emit_outproj(nb)

    if split_waits:
        split_multi_waits()
    return nc


def get_program(split_waits=True):
    key = (split_waits,)
    if key not in _PROGRAM:
        _PROGRAM[key] = _build_program(split_waits)
    return _PROGRAM[key]


def make_core_inputs(x, Wq, Wkv_a, Wkv_b, Wo):
    """Host-side sharding/pre-processing. Returns list of 8 input dicts."""
    scale = 1.0 / math.sqrt(DN + DR)

    inv_freq = 1.0 / (ROPE_THETA ** (np.arange(0, DR, 2, dtype=np.float64) / DR))
    t = np.arange(S, dtype=np.float64)
    freqs = np.outer(t, inv_freq)                      # [S, 32]
    cos32 = np.cos(freqs).T.astype(np.float32)         # [32, S]
    sin32 = np.sin(freqs).T.astype(np.float32)
    cosf = np.tile(cos32, (4, 1)).astype(BF16)         # [128, S]
    sinf = np.tile(np.concatenate([-sin32, sin32], axis=0), (2, 1)).astype(BF16)

    row = np.arange(128)[:, None]
    col = np.arange(128)[None, :]
    masks = np.where(col >= row, 0.0, -1e30).astype(BF16)  # [128, 128]
    ident = np.eye(128, dtype=BF16)
    ones = np.ones([128, 1], dtype=BF16)
    onesf = np.ones([1, 128], dtype=BF16)

    Wq_r = np.asarray(Wq, dtype=np.float32).reshape(D_MODEL, N_HEADS, DN + DR)
    Wb_r = np.asarray(Wkv_b, dtype=np.float32).reshape(R, N_HEADS, DN + DV)
    Wo_f = np.asarray(Wo, dtype=np.float32)
    Wkva_f = np.asarray(Wkv_a, dtype=np.float32)
    x_f = np.asarray(x, dtype=np.float32)

    in_maps = []
    for c in range(NCORES):
        b, g = divmod(c, HP)
        heads = list(range(HP * g, HP * g + HP))
        # chunk-major xT: block (t, k) contiguous [128, 512]
        xTc = np.ascontiguousarray(
            x_f[b].T.reshape(16, 128, NQB, QB).transpose(2, 0, 1, 3)
            .reshape(NQB * 16 * 128, QB)).astype(BF16)
        # fp8 pair-major x for the DoubleRow q projection.
        xT8c = np.ascontiguousarray(
            (x_f[b].T * SXQ).reshape(8, 2, 128, NQB, QB)
            .transpose(3, 0, 2, 1, 4)
            .reshape(NQB * 8 * 128, 2 * QB)).astype(F8E4)
        wq_nope = Wq_r[:, heads, :DN].reshape(D_MODEL, HP * DN)
        wq_rope = Wq_r[:, heads, DN:].reshape(D_MODEL, HP * DR)
        wq_c = np.concatenate([wq_nope, wq_rope], axis=1) * (scale * SWQ)
        wq8_c = np.ascontiguousarray(
            wq_c.reshape(8, 2, 128, HP * (DN + DR))
            .transpose(0, 2, 1, 3)
            .reshape(8 * 128, 2 * HP * (DN + DR))).astype(F8E4)
        wkva_c = np.ascontiguousarray(np.concatenate(
            [Wkva_f, Wkva_f[:, R:]], axis=1)).astype(BF16)
        wbk_c = np.ascontiguousarray(
            Wb_r[:, heads, :DN].reshape(R, HP * DN)).astype(BF16)
        wbv_c = np.ascontiguousarray(
            Wb_r[:, heads, DN:].reshape(R, HP * DV)).astype(BF16)
        wo_c = np.ascontiguousarray(
            Wo_f[HP * g * DV:(HP * g + HP) * DV, :]).astype(BF16)
        in_maps.append({
            "xT": xTc,
            "xT8": xT8c,
            "wq8": wq8_c,
            "wkva": wkva_c,
            "wkvbk": wbk_c,
            "wkvbv": wbv_c,
            "wo": wo_c,
            "cosf": cosf,
            "sinf": sinf,
            "masks": masks,
            "ident": ident,
            "ones": ones,
            "onesf": onesf,
        })
    return in_maps


def gather_output(results):
    """results: list of 8 dicts with 'outp' block-major bf16 partials."""
    out = np.empty((B, S, D_MODEL), dtype=np.float32)
    for b in range(B):
        acc = results[HP * b]["outp"].astype(np.float32)
        for g in range(1, HP):
            acc += results[HP * b + g]["outp"].astype(np.float32)
        # blocks (m, nb) -> [D_MODEL, S] -> transpose to [S, D_MODEL]
        out[b] = (acc.reshape(16, NQB, 128, QB).transpose(0, 2, 1, 3)
                  .reshape(D_MODEL, S).T)
    return out


def kernel(x, Wq, Wkv_a, Wkv_b, Wo):
    from concourse.bass_utils import run_bass_kernel_spmd

    nc = get_program()
    in_maps = make_core_inputs(x, Wq, Wkv_a, Wkv_b, Wo)
    res = run_bass_kernel_spmd(nc, in_maps, list(range(NCORES)))
    return gather_output(res.results)


# revision 23
# speedup vs baseline: 1.1771x; 1.0643x over previous
"""Multi-Head Latent Attention (MLA) TRN2 Bass kernel.

Sharding: data-parallel over batch (B=2) x tensor-parallel over heads
(16 heads -> 4 per core) = 8 cores. The kv_lora latent path and shared
rope key are computed replicated within each batch group (cross-core
AllGather / Pool-engine offload both trip the chip power throttle and
net out slower -- measured); the final output projection is computed as
per-core partials which the host sums.

Structure (two phases, internally pipelined; fully merging the phases
was measured SLOWER -- cross-engine SBUF/PSUM contention inflates every
matmul by 5-20%):
  Phase 1, per 512-column quarter t: kv_a (bf16) -> q-proj (fp8
  DoubleRow; x and Wq pre-scaled to fp8 on host) -> kv_b (k_nopeT, v)
  -> RoPE -> fp8 casts -> DoubleRow packing. The rope/cast/pack engine
  work of quarter t overlaps quarter t+1's PE matmuls, and attention
  can start right after the last quarter (its first q-block only needs
  quarter-0 data).
  Phase 2: attention units iterated q-block-major ((h, j) with j outer)
  with the output projection of column block j interleaved after the
  second unit of block j+1 -- out-proj is PE-heavy/ACT-light, so it
  fills the PE gaps left by the ACT-bound exp pipeline, and the output
  DMA spreads across the phase instead of draining at the end.

Scores per (head, q-block) via one fp8e4 DoubleRow matmul per k-chunk
(the 192-dim nope+rope contraction packed as 2x96 partitions, q/k
pre-scaled by 64/32 and descaled inside the exp). Chunk PAIRS share one
2-bank PSUM tile; the second chunk's columns are SHIFTED to start at
column 512 of the tile regardless of causal trim, so the pair's exp
input is one contiguous gap-free span (one ACT op, no stale bytes,
minimum width). Causal masks are added on diagonal chunks by a bf16 PE
matmul; exp on ACT with no max subtraction (scores are bounded);
softmax denominators via running elementwise bf16 sums of the exp'd
chunks on the DVE + one ones-matmul per unit, deferred one unit so the
PE never stalls on them; outT accumulated in PSUM and normalized by
broadcasted reciprocals.

x and outp use chunk-major DRAM layouts so every DMA is one dense
contiguous block (host packs/unpacks); outp is bf16 partials summed in
f32 on the host.
"""

import math
import sys

import numpy as np
import ml_dtypes

try:  # concourse ships in the container; fall back to the repo checkout
    import concourse.bass  # noqa: F401
except ImportError:  # pragma: no cover
    for p in ("/opt/trn_rl_repo", "/root/.axon_site/_ro/trn_rl_repo"):
        if p not in sys.path:
            sys.path.insert(0, p)

# Problem constants (hardcoded; harness calls kernel() standalone).
D_MODEL = 2048
N_HEADS = 16
R = 512          # kv lora rank
DN = 128         # d_nope
DR = 64          # d_rope
DV = 128         # d_v
ROPE_THETA = 10000.0
B = 2
S = 2048
HP = 4           # heads per core
QB = 512         # q block size
NKC = S // 128   # 16 k chunks
NQB = S // QB    # 4 q blocks
NCORES = 8

BF16 = ml_dtypes.bfloat16
F8E4 = ml_dtypes.float8_e4m3  # mybir float8e4 (IEEE e4m3, max finite 240)

# fp8 pre-scales. QS/KS: q/k tiles feeding the score matmuls (descaled
# inside the exp). SXQ/SWQ: host-side scales for x / Wq feeding the fp8
# DoubleRow q-projection (descaled in the PSUM->SBUF store).
QS, KS = 64.0, 32.0
SXQ, SWQ = 16.0, 8192.0

_PROGRAM = {}


def _build_program(split_waits=True):
    import concourse.bass as bass
    import concourse.bass_isa as bass_isa
    import concourse.mybir as mybir
    from concourse.tile import TileContext

    def split_multi_waits(max_waits=1):
        """The walrus build in this container rejects instructions with
        more than `max_waits` sync-wait commands. Move excess waits onto
        same-engine NoOps inserted just before the instruction."""
        for f in nc.m.functions:
            for bb in f.blocks:
                out = []
                changed = False
                for inst in bb.instructions:
                    si = getattr(inst, "sync_info", None)
                    ws = list(si.on_wait) if si is not None else []
                    is_pool = getattr(inst, "engine", None) == \
                        mybir.EngineType.Pool
                    if len(ws) > max_waits and not is_pool:
                        changed = True
                        inst.sync_info = mybir.SyncInfo(
                            on_wait=ws[:max_waits],
                            on_update=list(si.on_update))
                        for w in ws[max_waits:]:
                            n = mybir.InstNoOp(
                                name=nc.get_next_instruction_name(),
                                ins=[], outs=[])
                            n.engine = inst.engine
                            n.sync_info = mybir.SyncInfo(
                                on_wait=[w], on_update=[])
                            out.append(n)
                    out.append(inst)
                if changed:
                    bb.instructions = out

    f32 = mybir.dt.float32
    cdt = mybir.dt.bfloat16
    f8 = mybir.dt.float8e4
    DRmode = mybir.MatmulPerfMode.DoubleRow
    Copy = mybir.ActivationFunctionType.Copy
    Exp = mybir.ActivationFunctionType.Exp
    Ln = mybir.ActivationFunctionType.Ln

    nc = bass.Bass()

    # x arrives chunk-major: block (t, k) = xT_logical[128k:128k+128,
    # 512t:512t+512] stored contiguously so every chunk DMA is one dense
    # 128KB transfer instead of 128 separate 1KB rows.
    xT = nc.dram_tensor("xT", [NQB * 16 * 128, QB], cdt, kind="ExternalInput")
    # fp8 copy of x for the q-projection, pair-major for DoubleRow:
    # block (t, p) = [128, 2, 512]: elem (kp, j, col) =
    # SXQ * x[512t+col, 256p+128j+kp] (feature on partitions).
    xT8 = nc.dram_tensor("xT8", [NQB * 8 * 128, 2 * QB], f8,
                         kind="ExternalInput")
    # Wq in fp8, pair-major: block p = [128, 2, 768]: elem (kp, j, m) =
    # SWQ * scale * Wq[256p+128j+kp, m] (m = head-major nope|rope cols).
    wq8 = nc.dram_tensor("wq8", [8 * 128, 2 * HP * (DN + DR)], f8,
                         kind="ExternalInput")
    # wkva cols: 512 latent | 64 rope | 64 rope again (duplicated so the
    # rope projection runs as one full 128-wide matmul and lands already
    # row-duplicated for the swap-rope layout).
    wkva = nc.dram_tensor("wkva", [D_MODEL, R + 2 * DR], cdt,
                          kind="ExternalInput")
    wkvbk = nc.dram_tensor("wkvbk", [R, HP * DN], cdt, kind="ExternalInput")
    wkvbv = nc.dram_tensor("wkvbv", [R, HP * DV], cdt, kind="ExternalInput")
    wo = nc.dram_tensor("wo", [HP * DV, D_MODEL], cdt, kind="ExternalInput")
    cosf = nc.dram_tensor("cosf", [128, S], cdt, kind="ExternalInput")
    sinf = nc.dram_tensor("sinf", [128, S], cdt, kind="ExternalInput")
    masks = nc.dram_tensor("masks", [128, 128], cdt, kind="ExternalInput")
    ident = nc.dram_tensor("ident", [128, 128], cdt, kind="ExternalInput")
    ones = nc.dram_tensor("ones", [128, 1], cdt, kind="ExternalInput")
    onesf = nc.dram_tensor("onesf", [1, 128], cdt, kind="ExternalInput")
    # outp is block-major: block (m, nb) stored contiguously; host unpacks.
    outp = nc.dram_tensor("outp", [16 * NQB * 128, QB], cdt,
                          kind="ExternalOutput")

    with TileContext(nc) as tc:
        with (
            tc.tile_pool(name="const", bufs=1) as cpool,
            tc.tile_pool(name="persist", bufs=1) as ppool,
        ):
            cosf_sb = cpool.tile([128, S], cdt, name="cosf_sb")
            sinf_sb = cpool.tile([128, S], cdt, name="sinf_sb")
            masks_sb = cpool.tile([128, 128], cdt, name="masks_sb")
            ident_sb = cpool.tile([128, 128], cdt, name="ident_sb")
            ones_sb = cpool.tile([128, 1], cdt, name="ones_sb")
            onesb_sb = cpool.tile([1, 128], cdt, name="onesb_sb")

            # Persistent activations. q_nope / k_nope tiles live in fp8
            # (written pre-scaled straight from PSUM); rope halves stay
            # bf16 until after the RoPE rotation, then are cast.
            qT8 = [
                ppool.tile([128, S], f8, name=f"qT8_{m}", tag="qT8", bufs=4)
                for m in range(4)
            ]
            qTr = [
                ppool.tile([128, S], cdt, name=f"qTr{m}", tag="qT", bufs=2)
                for m in range(2)
            ]
            kn8 = [
                ppool.tile([128, S], f8, name=f"kn8_{m}", tag="kn8", bufs=4)
                for m in range(4)
            ]
            qr8 = [
                ppool.tile([128, S], f8, name=f"qr8_{m}", tag="qr8", bufs=2)
                for m in range(2)
            ]
            kr8 = ppool.tile([128, S], f8, name="kr8", tag="kr8", bufs=1)
            # DoubleRow-packed per-head tensors: 192 contraction dims
            # as 2 blocks of 96 partitions (blk0 = dims 0:96,
            # blk1 = dims 96:192 = nope 96:128 + rope 0:64).
            qf8 = [
                ppool.tile([96, NQB, 2, QB], f8, name=f"qf8_{h}", tag="qf8",
                           bufs=4)
                for h in range(HP)
            ]
            kf8 = [
                ppool.tile([96, NKC, 2, 128], f8, name=f"kf8_{h}", tag="kf8",
                           bufs=4)
                for h in range(HP)
            ]
            ck = [
                ppool.tile([128, S], cdt, name=f"ck{m}", tag="cko", bufs=4)
                for m in range(4)
            ]
            kr = ppool.tile([128, S], cdt, name="krope", tag="krope", bufs=1)
            vt = [
                ppool.tile([128, HP * DV], cdt, name=f"v{i}", tag="v",
                           bufs=NKC)
                for i in range(NKC)
            ]
            def store_q(m, cols, ps):
                # PSUM -> SBUF store for qT M-tile m, descaling the host
                # fp8 pre-scales (SXQ*SWQ); nope tiles also pick up the
                # QS score pre-scale and go straight to fp8.
                if m < 4:
                    nc.scalar.activation(qT8[m][:, cols], ps, Copy,
                                         scale=QS / (SXQ * SWQ))
                else:
                    nc.scalar.activation(qTr[m - 4][:, cols], ps, Copy,
                                         scale=1.0 / (SXQ * SWQ))

            # kv_b weights persist so their DMAs can issue at startup.
            wbk_sb = [
                ppool.tile([128, HP * DN], cdt, name=f"wbk_sb{r}", tag="wbk",
                           bufs=4)
                for r in range(4)
            ]
            wbv_sb = [
                ppool.tile([128, HP * DV], cdt, name=f"wbv_sb{r}", tag="wbv",
                           bufs=4)
                for r in range(4)
            ]

            # ---- Phase 1: per-quarter projections + rope + packing ----
            with (
                tc.tile_pool(name="wproj", bufs=1) as wpool,
                tc.tile_pool(name="xstream", bufs=1) as xpool,
                tc.tile_pool(name="psA", bufs=8, space="PSUM") as psA,
            ):
                # Weight DMAs on the scalar queue; wkva first (the very
                # first PE matmul needs wkva_sb[0]), wq8 interleaved.
                wkva_sb = []
                wq8_sb = []
                for k in range(16):
                    w2 = wpool.tile([128, R + 2 * DR], cdt,
                                    name=f"wkva_sb{k}", tag="wkva", bufs=16)
                    nc.scalar.dma_start(w2, wkva[k * 128:(k + 1) * 128, :])
                    wkva_sb.append(w2)
                    if k < 8:
                        w1 = wpool.tile([128, 2, HP * (DN + DR)], f8,
                                        name=f"wq8_sb{k}", tag="wq8", bufs=8)
                        nc.scalar.dma_start(w1, wq8[k * 128:(k + 1) * 128, :])
                        wq8_sb.append(w1)
                for r in range(4):
                    nc.scalar.dma_start(wbk_sb[r],
                                        wkvbk[r * 128:(r + 1) * 128, :])
                    nc.scalar.dma_start(wbv_sb[r],
                                        wkvbv[r * 128:(r + 1) * 128, :])
                nc.scalar.dma_start(cosf_sb, cosf[:, :])
                nc.scalar.dma_start(sinf_sb, sinf[:, :])
                nc.scalar.dma_start(masks_sb, masks[:, :])
                nc.scalar.dma_start(ident_sb, ident[:, :])
                nc.scalar.dma_start(ones_sb, ones[:, :])
                nc.scalar.dma_start(onesb_sb, onesf[:, :])

                for t in range(NQB):
                    tcols = slice(t * QB, (t + 1) * QB)
                    xq = []
                    xq8t = []
                    for k in range(16):
                        xk = xpool.tile([128, QB], cdt, name=f"xq_{t}_{k}",
                                        tag="xq", bufs=24)
                        nc.sync.dma_start(
                            xk,
                            xT[(t * 16 + k) * 128:(t * 16 + k + 1) * 128, :])
                        xq.append(xk)
                        if k % 2 == 1:
                            p = k // 2
                            x8 = xpool.tile([128, 2, QB], f8,
                                            name=f"xq8_{t}_{p}", tag="xq8",
                                            bufs=16)
                            nc.sync.dma_start(
                                x8,
                                xT8[(t * 8 + p) * 128:(t * 8 + p + 1) * 128,
                                    :])
                            xq8t.append(x8)
                    # kv_a first (bf16, DMA-latency friendly at t=0): each
                    # landing x chunk feeds 5 matmuls.
                    ps_k = [
                        psA.tile([128, QB], f32, name=f"psk_{t}_{m}",
                                 tag="ps")
                        for m in range(4)
                    ]
                    ps_r = psA.tile([128, QB], f32, name=f"psr_{t}",
                                    tag="ps")
                    for k in range(16):
                        for m in range(4):
                            nc.tensor.matmul(
                                ps_k[m],
                                lhsT=wkva_sb[k][:, m * 128:(m + 1) * 128],
                                rhs=xq[k], start=(k == 0), stop=(k == 15))
                        nc.tensor.matmul(
                            ps_r, lhsT=wkva_sb[k][:, R:R + 2 * DR],
                            rhs=xq[k], start=(k == 0), stop=(k == 15))
                    for m in range(4):
                        nc.vector.tensor_copy(ck[m][:, tcols], ps_k[m])
                    nc.scalar.copy(kr[:, tcols], ps_r)
                    # q projection: fp8 DoubleRow, 2 chunks per matmul.
                    ps_q = [
                        psA.tile([128, QB], f32, name=f"psq_{t}_{m}",
                                 tag="ps")
                        for m in range(6)
                    ]
                    for p in range(8):
                        for m in range(6):
                            nc.tensor.matmul(
                                ps_q[m],
                                lhsT=wq8_sb[p][:, :, m * 128:(m + 1) * 128],
                                rhs=xq8t[p], start=(p == 0), stop=(p == 7),
                                perf_mode=DRmode)
                    for m in range(6):
                        store_q(m, tcols, ps_q[m])
                    # kv up-projection for this quarter: k_nopeT + v.
                    for m in range(4):
                        ps = psA.tile([128, QB], f32, name=f"psn_{t}_{m}",
                                      tag="ps")
                        for r in range(4):
                            nc.tensor.matmul(
                                ps, lhsT=wbk_sb[r][:, m * 128:(m + 1) * 128],
                                rhs=ck[r][:, tcols], start=(r == 0),
                                stop=(r == 3))
                        nc.scalar.activation(kn8[m][:, tcols], ps, Copy,
                                             scale=KS)
                    for ci in range(4):
                        c = 4 * t + ci
                        ps = psA.tile([128, HP * DV], f32, name=f"psv_{c}",
                                      tag="ps")
                        for r in range(4):
                            nc.tensor.matmul(
                                ps, lhsT=ck[r][:, c * 128:(c + 1) * 128],
                                rhs=wbv_sb[r], start=(r == 0), stop=(r == 3))
                        nc.vector.tensor_copy(vt[c], ps)
                    # RoPE for this quarter (DVE + swap DMAs), fp8 casts,
                    # and DoubleRow packing -- all overlap quarter t+1's
                    # PE matmuls.
                    for idx, tapt in enumerate([qTr[0], qTr[1], kr]):
                        sw = ppool.tile([128, QB], cdt, name=f"sw_{t}_{idx}",
                                        tag="sw", bufs=3)
                        for blk in range(4):
                            src = (blk ^ 1) * 32
                            nc.sync.dma_start(
                                sw[blk * 32:(blk + 1) * 32, :],
                                tapt[src:src + 32, tcols])
                        tap = tapt[:, tcols]
                        nc.vector.tensor_mul(tap, tap, cosf_sb[:, tcols])
                        nc.vector.tensor_mul(sw, sw, sinf_sb[:, tcols])
                        nc.vector.tensor_add(tap, tap, sw)
                    nc.scalar.activation(qr8[0][:, tcols], qTr[0][:, tcols],
                                         Copy, scale=QS)
                    nc.scalar.activation(qr8[1][:, tcols], qTr[1][:, tcols],
                                         Copy, scale=QS)
                    nc.scalar.activation(kr8[:, tcols], kr[:, tcols],
                                         Copy, scale=KS)
                    for h in range(HP):
                        off = (h % 2) * 64
                        ri = h // 2
                        c4 = slice(4 * t, 4 * t + 4)
                        nc.sync.dma_start(qf8[h][0:96, t, 0, :],
                                          qT8[h][0:96, tcols])
                        nc.sync.dma_start(qf8[h][0:32, t, 1, :],
                                          qT8[h][96:128, tcols])
                        nc.sync.dma_start(qf8[h][32:96, t, 1, :],
                                          qr8[ri][off:off + 64, tcols])
                        nc.scalar.dma_start(kf8[h][0:96, c4, 0, :],
                                            kn8[h][0:96, tcols])
                        nc.scalar.dma_start(kf8[h][0:32, c4, 1, :],
                                            kn8[h][96:128, tcols])
                        nc.scalar.dma_start(kf8[h][32:96, c4, 1, :],
                                            kr8[off:off + 64, tcols])

            # outT tiles reuse the c_kvT slots (same tag, 4 bufs); ck is
            # fully consumed by the per-quarter kv_b above.
            outT = [
                ppool.tile([128, S], cdt, name=f"outT{h}", tag="cko", bufs=4)
                for h in range(HP)
            ]

            # ---- Phase 2: attention with interleaved output projection ----
            with (
                tc.tile_pool(name="att", bufs=1) as apool,
                tc.tile_pool(name="psS", bufs=4, space="PSUM") as psS,
                tc.tile_pool(name="psO", bufs=2, space="PSUM") as psO,
                tc.tile_pool(name="psD", bufs=1, space="PSUM") as psD,
                tc.tile_pool(name="psBC", bufs=1, space="PSUM") as psBC,
            ):
                wo_sb = [
                    apool.tile([128, D_MODEL], cdt, name=f"wo_sb{r}",
                               tag="wo", bufs=4)
                    for r in range(4)
                ]
                for r in range(4):
                    nc.sync.dma_start(wo_sb[r], wo[r * 128:(r + 1) * 128, :])

                def calc_dps(h, j, ptacc):
                    # Single partition-sum matmul over the accumulated
                    # exp'd chunks (deferred off the critical path). The
                    # GPSIMD partition_all_reduce variant is unbuildable
                    # here (this walrus has no Pool custom-ISA lowering).
                    dps = psD.tile([1, QB], f32, name=f"dps_{h}_{j}",
                                   tag="d")
                    nc.tensor.matmul(dps, lhsT=ones_sb, rhs=ptacc,
                                     start=True, stop=True)
                    return dps

                def norm_early(h, j, dps):
                    # 1/denom as exp(-ln(d)) on the ACT engine.
                    rec = apool.tile([1, QB], f32, name=f"rec_{h}_{j}",
                                     tag="rec", bufs=2)
                    nc.scalar.activation(rec, dps, Ln)
                    recb = apool.tile([1, QB], cdt, name=f"recb_{h}_{j}",
                                      tag="recb", bufs=2)
                    nc.scalar.activation(recb, rec, Exp, scale=-1.0)
                    return recb

                def norm_late(h, j, ops, recb):
                    # Broadcast 1/denom across partitions via a K=1
                    # matmul, then scale the out accumulator into outT.
                    qs = slice(j * QB, (j + 1) * QB)
                    bps = psBC.tile([128, QB], f32, name=f"bps_{h}_{j}",
                                    tag="b")
                    nc.tensor.matmul(bps, lhsT=onesb_sb, rhs=recb,
                                     start=True, stop=True)
                    bc = apool.tile([128, QB], f32, name=f"bc_{h}_{j}",
                                    tag="bc", bufs=2)
                    nc.scalar.copy(bc, bps)
                    nc.vector.tensor_mul(outT[h][:, qs], ops, bc)

                def emit_unit(h, j, pend):
                    """One (head, q-block) attention unit with the
                    deferred-normalization pipeline (previous unit's
                    denominator at pair 0, normalize at pair 1)."""
                    ops = psO.tile([128, QB], f32, name=f"ops_{h}_{j}",
                                   tag="o")
                    ptacc = apool.tile([128, QB], cdt, name=f"pta_{h}_{j}",
                                       tag="pta", bufs=2)
                    nch = 4 * (j + 1)
                    for p in range(nch // 2):
                        cA, cB = 2 * p, 2 * p + 1
                        rA, rB = cA - 4 * j, cB - 4 * j
                        colA = max(0, rA * 128)
                        colB = max(0, rB * 128)
                        bw = QB - colB  # B half is shifted to start at QB
                        pairps = psS.tile([128, 2 * QB], f32,
                                          name=f"sps_{h}_{j}_{p}", tag="s",
                                          bufs=2)
                        nc.tensor.matmul(
                            pairps[:, colA:QB],
                            lhsT=kf8[h][:, cA, :, :],
                            rhs=qf8[h][:, j, :, colA:QB],
                            start=True, stop=(rA < 0),
                            perf_mode=DRmode, skip_group_check=True)
                        if rA >= 0:
                            nc.tensor.matmul(
                                pairps[:, colA:colA + 128],
                                lhsT=ident_sb, rhs=masks_sb,
                                start=False, stop=True,
                                skip_group_check=True)
                        nc.tensor.matmul(
                            pairps[:, QB:QB + bw],
                            lhsT=kf8[h][:, cB, :, :],
                            rhs=qf8[h][:, j, :, colB:QB],
                            start=True, stop=(rB < 0),
                            perf_mode=DRmode, skip_group_check=True)
                        if rB >= 0:
                            nc.tensor.matmul(
                                pairps[:, QB:QB + 128],
                                lhsT=ident_sb, rhs=masks_sb,
                                start=False, stop=True,
                                skip_group_check=True)
                        # One contiguous exp over both halves.
                        ptp = apool.tile([128, 2 * QB], cdt,
                                         name=f"pt_{h}_{j}_{p}", tag="pt",
                                         bufs=2)
                        nc.scalar.activation(
                            ptp[:, colA:QB + bw], pairps[:, colA:QB + bw],
                            Exp, scale=1.0 / (QS * KS))
                        nc.tensor.matmul(
                            ops[:, colA:QB],
                            lhsT=vt[cA][:, h * DV:(h + 1) * DV],
                            rhs=ptp[:, colA:QB], start=(cA == 0),
                            stop=False, skip_group_check=True)
                        nc.tensor.matmul(
                            ops[:, colB:QB],
                            lhsT=vt[cB][:, h * DV:(h + 1) * DV],
                            rhs=ptp[:, QB:QB + bw],
                            start=False, stop=(cB == nch - 1),
                            skip_group_check=True)
                        if cA == 0:
                            nc.vector.tensor_copy(ptacc, ptp[:, 0:QB])
                        else:
                            nc.vector.tensor_add(
                                ptacc[:, colA:QB], ptacc[:, colA:QB],
                                ptp[:, colA:QB])
                        nc.vector.tensor_add(
                            ptacc[:, colB:QB], ptacc[:, colB:QB],
                            ptp[:, QB:QB + bw])
                        if p == 0 and pend is not None and len(pend) == 4:
                            ph, pj, pops, pacc = pend
                            recb = norm_early(ph, pj,
                                              calc_dps(ph, pj, pacc))
                            pend = (ph, pj, pops, recb, True)
                        if p == 1 and pend is not None and len(pend) == 5:
                            ph, pj, pops, recb, _ = pend
                            norm_late(ph, pj, pops, recb)
                            pend = None
                    return (h, j, ops, ptacc)

                def emit_outproj(nb):
                    """Output projection for column block nb. Alternate
                    between the score-pair and out-accumulator PSUM slots
                    (effective ring depth 4) so the staging-copy latency
                    never stalls the PE; each [128,512] block DMAs out as
                    soon as its copy lands."""
                    ncols = slice(nb * QB, (nb + 1) * QB)
                    for m in range(16):
                        if m % 2 == 0:
                            ps = psS.tile([128, 2 * QB], f32,
                                          name=f"pw_{nb}_{m}", tag="s",
                                          bufs=2)[:, 0:QB]
                        else:
                            ps = psO.tile([128, QB], f32,
                                          name=f"pw_{nb}_{m}", tag="o")
                        for r in range(4):
                            nc.tensor.matmul(
                                ps, lhsT=wo_sb[r][:, m * 128:(m + 1) * 128],
                                rhs=outT[r][:, ncols], start=(r == 0),
                                stop=(r == 3), skip_group_check=True)
                        st = apool.tile([128, QB], cdt, name=f"st_{nb}_{m}",
                                        tag="st", bufs=4)
                        if m % 2 == 0:
                            nc.scalar.copy(st, ps)
                        else:
                            nc.vector.tensor_copy(st, ps)
                        eng = nc.sync if m % 2 == 0 else nc.scalar
                        eng.dma_start(
                            outp[(m * NQB + nb) * 128:
                                 (m * NQB + nb + 1) * 128, :], st)

                pend = None
                for j in range(NQB):
                    for h in range(HP):
                        pend = emit_unit(h, j, pend)
                # Final flush, then the output projection as one clean
                # PE-bound block (interleaving it with the ACT-bound
                # attention units was measured slower: cross-engine
                # contention taxes every matmul ~15%).
                ph, pj, pops, pacc = pend
                recb = norm_early(ph, pj, calc_dps(ph, pj, pacc))
                norm_late(ph, pj, pops, recb)
                for nb in range(NQB):
                    emit_outproj(nb)

    if split_waits:
        split_multi_waits()
    return nc


def get_program(split_waits=True):
    key = (split_waits,)
    if key not in _PROGRAM:
        _PROGRAM[key] = _build_program(split_waits)
    return _PROGRAM[key]


def make_core_inputs(x, Wq, Wkv_a, Wkv_b, Wo):
    """Host-side sharding/pre-processing. Returns list of 8 input dicts."""
    scale = 1.0 / math.sqrt(DN + DR)

    inv_freq = 1.0 / (ROPE_THETA ** (np.arange(0, DR, 2, dtype=np.float64) / DR))
    t = np.arange(S, dtype=np.float64)
    freqs = np.outer(t, inv_freq)                      # [S, 32]
    cos32 = np.cos(freqs).T.astype(np.float32)         # [32, S]
    sin32 = np.sin(freqs).T.astype(np.float32)
    cosf = np.tile(cos32, (4, 1)).astype(BF16)         # [128, S]
    sinf = np.tile(np.concatenate([-sin32, sin32], axis=0), (2, 1)).astype(BF16)

    row = np.arange(128)[:, None]
    col = np.arange(128)[None, :]
    masks = np.where(col >= row, 0.0, -1e30).astype(BF16)  # [128, 128]
    ident = np.eye(128, dtype=BF16)
    ones = np.ones([128, 1], dtype=BF16)
    onesf = np.ones([1, 128], dtype=BF16)

    Wq_r = np.asarray(Wq, dtype=np.float32).reshape(D_MODEL, N_HEADS, DN + DR)
    Wb_r = np.asarray(Wkv_b, dtype=np.float32).reshape(R, N_HEADS, DN + DV)
    Wo_f = np.asarray(Wo, dtype=np.float32)
    Wkva_f = np.asarray(Wkv_a, dtype=np.float32)
    x_f = np.asarray(x, dtype=np.float32)

    in_maps = []
    for c in range(NCORES):
        b, g = divmod(c, HP)
        heads = list(range(HP * g, HP * g + HP))
        # chunk-major xT: block (t, k) contiguous [128, 512]
        xTc = np.ascontiguousarray(
            x_f[b].T.reshape(16, 128, NQB, QB).transpose(2, 0, 1, 3)
            .reshape(NQB * 16 * 128, QB)).astype(BF16)
        # fp8 pair-major x for the DoubleRow q projection.
        xT8c = np.ascontiguousarray(
            (x_f[b].T * SXQ).reshape(8, 2, 128, NQB, QB)
            .transpose(3, 0, 2, 1, 4)
            .reshape(NQB * 8 * 128, 2 * QB)).astype(F8E4)
        wq_nope = Wq_r[:, heads, :DN].reshape(D_MODEL, HP * DN)
        wq_rope = Wq_r[:, heads, DN:].reshape(D_MODEL, HP * DR)
        wq_c = np.concatenate([wq_nope, wq_rope], axis=1) * (scale * SWQ)
        wq8_c = np.ascontiguousarray(
            wq_c.reshape(8, 2, 128, HP * (DN + DR))
            .transpose(0, 2, 1, 3)
            .reshape(8 * 128, 2 * HP * (DN + DR))).astype(F8E4)
        wkva_c = np.ascontiguousarray(np.concatenate(
            [Wkva_f, Wkva_f[:, R:]], axis=1)).astype(BF16)
        wbk_c = np.ascontiguousarray(
            Wb_r[:, heads, :DN].reshape(R, HP * DN)).astype(BF16)
        wbv_c = np.ascontiguousarray(
            Wb_r[:, heads, DN:].reshape(R, HP * DV)).astype(BF16)
        wo_c = np.ascontiguousarray(
            Wo_f[HP * g * DV:(HP * g + HP) * DV, :]).astype(BF16)
        in_maps.append({
            "xT": xTc,
            "xT8": xT8c,
            "wq8": wq8_c,
            "wkva": wkva_c,
            "wkvbk": wbk_c,
            "wkvbv": wbv_c,
            "wo": wo_c,
            "cosf": cosf,
            "sinf": sinf,
            "masks": masks,
            "ident": ident,
            "ones": ones,
            "onesf": onesf,
        })
    return in_maps


def gather_output(results):
    """results: list of 8 dicts with 'outp' block-major bf16 partials."""
    out = np.empty((B, S, D_MODEL), dtype=np.float32)
    for b in range(B):
        acc = results[HP * b]["outp"].astype(np.float32)
        for g in range(1, HP):
            acc += results[HP * b + g]["outp"].astype(np.float32)
        # blocks (m, nb) -> [D_MODEL, S] -> transpose to [S, D_MODEL]
        out[b] = (acc.reshape(16, NQB, 128, QB).transpose(0, 2, 1, 3)
                  .reshape(D_MODEL, S).T)
    return out


def kernel(x, Wq, Wkv_a, Wkv_b, Wo):
    from concourse.bass_utils import run_bass_kernel_spmd

    nc = get_program()
    in_maps = make_core_inputs(x, Wq, Wkv_a, Wkv_b, Wo)
    res = run_bass_kernel_spmd(nc, in_maps, list(range(NCORES)))
    return gather_output(res.results)
